# revision 74
# baseline (speedup 1.0000x reference)
"""Causal3DTransformerBlock on 8 TRN2 NeuronCores.

Sharding: self-attention is head-parallel with ONE head x BOTH batches per
core (core h owns head h).  A single 8-core AllToAll redistributes the
attention outputs to token-parallel (core j: batch j//4, tokens
(j%4)*512 .. +512); every A2A block is real data and the gathered rows are
head-major on every core, so the out-projection (full C contraction) runs
after the exchange with the unpermuted w_attn_out.  Cross-attention and the
SwiGLU FFN are token-parallel (no further collectives).

Compute dtype bf16 (fp32 PSUM accumulation, fp32 residual stream).
LayerNorm gains fold into weights host-side; mean/bias enter as a K=2
rank-1 correction matmul, except the FFN where the mean is subtracted on
DVE (the bias path compiles only when ln3_b is nonzero).
Softmax: raw exp (1/sqrt(dh) folded into wq/wk), denominator accumulated
by an extra ones-row in V, normalization via DVE fast reciprocal +
ones-matmul broadcast.  The FFN runs in fp8 (e4m3, weights x64) with
DoubleRow matmuls; SiLU on the ACT engine descales the gate.
Large weights (wqc, wo, wco, wg, wu, wd) are host-pre-tiled and streamed
through small double-buffered SBUF tiles.
"""

import sys

sys.path.insert(0, "/opt/trn_rl_repo")

import os

import numpy as np
import ml_dtypes

BF16 = ml_dtypes.bfloat16
SKIP_COLLECTIVE = bool(int(os.environ.get("K_SKIP_COLLECTIVE", "0")))

B, S, C, CTX, II, H, DH = 2, 2048, 768, 128, 3072, 8, 96
NCORES = 8
TG = 512         # tokens per core after the exchange (A2A block width)
MG = 1024        # megagroup width for self-attention phases
NMG = S // MG    # 2
NKT = S // 128   # 16 key tiles
KPG = MG // 128  # 8 key tiles per megagroup
NCT = C // 128   # 6 feature tiles
NIT = II // 128  # 24 FFN intermediate tiles
DP = 128         # stored (permuted+padded) Q/K head dim
EPS = 1e-5
RG8 = [[0, 1, 2, 3, 4, 5, 6, 7]]

_CACHE = {}


def _build_program(bias_zero, debug=False):
    import concourse.bass as bass
    import concourse.tile as tile
    from concourse import bacc, mybir
    from concourse.alu_op_type import AluOpType as alu

    f32 = mybir.dt.float32
    bf16 = mybir.dt.bfloat16
    AF = mybir.ActivationFunctionType

    nc = bacc.Bacc("TRN2", debug=False, num_devices=NCORES)

    def din(name, shape, dt=bf16):
        return nc.dram_tensor(name, shape, dt, kind="ExternalInput").ap()

    # x^T pre-tiled [128, (g,c)-blocks of MG cols], both batches
    x_bf = [din(f"x_bf{b}", [128, NMG * NCT * MG]) for b in range(B)]
    x_own = din("x_own", [C, TG], f32)    # own 512-token slice, fp32
    ctx_bf = din("ctx_bf", [128, NCT * CTX])  # own batch ctx^T, pre-tiled
    cosT = din("cosT", [DP, S])
    sinT = din("sinT", [DP, S])           # sign-folded, partner-swapped (^64)
    tri = din("tri", [128, 128])          # triu {0,1} mask: [k,q] valid q>=k
    ones_in = din("ones_in", [128, 128])
    # merged [wq|wk|wv] pre-tiled: block c at cols c*(DP+DP+DH)
    wqkv = din("wqkv_t", [128, NCT * (2 * DP + DH)])
    wkc = din("wkc", [128, NCT * C])      # pre-tiled, block c at cols c*C
    wvc = din("wvc", [128, NCT * C])
    # pre-tiled [128, blocks]: see _prep_inputs for the layouts
    wo_t = din("wo_t", [128, NCT * NCT * 128])
    wqc_t = din("wqc_t", [128, H * NCT * DH])
    wco_t = din("wco_t", [128, NCT * NCT * 128])
    # FFN weights in fp8 (x64 scaled), DoubleRow-packed: two contraction
    # rows interleaved along the free dim; block (it, cpair) is 256 cols
    fp8 = mybir.dt.float8e4
    wg_t = din("wg_t", [128, NIT * 3 * 256], fp8)
    wu_t = din("wu_t", [128, NIT * 3 * 256], fp8)
    wd_t = din("wd_t", [128, NCT * 12 * 256], fp8)
    cq = din("cq", [2, DP])               # corrections: row0=-colsum(W'), row1=b@W
    ck = din("ck", [2, DP])
    cv = din("cv", [2, DH])
    cqc = din("cqc", [2, C])
    if not bias_zero:
        cg = din("cg", [1, II])           # b@Wg (bias variant only)
        cu = din("cu", [1, II])

    out_x = nc.dram_tensor("out_x", [C, TG], f32, kind="ExternalOutput").ap()
    if debug:
        dbg_ai = nc.dram_tensor("dbg_ai", [H * DH, TG], bf16,
                                kind="ExternalOutput").ap()
        dbg_ao = nc.dram_tensor("dbg_ao", [H * DH, TG], bf16,
                                kind="ExternalOutput").ap()
        dbg_x2 = nc.dram_tensor("dbg_x2", [C, TG], f32,
                                kind="ExternalOutput").ap()
        dbg_x3 = nc.dram_tensor("dbg_x3", [C, TG], f32,
                                kind="ExternalOutput").ap()

    with tile.TileContext(nc) as tc:
        with (
            tc.tile_pool(name="const", bufs=1) as cpool,
            tc.tile_pool(name="resid", bufs=1) as rpool,
            tc.tile_pool(name="work", bufs=2) as wpool,
            tc.tile_pool(name="stat", bufs=1) as spool,
            tc.tile_pool(name="dram", bufs=1, space="DRAM") as dpool,
        ):
            # ---- const APs for activation bias ----
            czero = cpool.tile([128, 1], f32, tag="czero", name="czero")
            nc.vector.memset(czero[:], 0.0)
            nc.const_aps.aps[(f32, 0.0)] = czero[:]
            ceps = cpool.tile([128, 1], f32, tag="ceps", name="ceps")
            nc.vector.memset(ceps[:], EPS)
            nc.const_aps.aps[(f32, EPS)] = ceps[:]
            ones_sb = cpool.tile([128, 128], bf16, tag="ones", name="ones")
            nc.sync.dma_start(ones_sb[:], ones_in)
            tri_sb = cpool.tile([128, 128], bf16, tag="tri", name="tri")
            nc.sync.dma_start(tri_sb[:], tri)


            def mmF(ps, lhsT, rhs, c0, c1, start, stop):
                """matmul on cols [c0:c1) of ps/rhs, split at the PSUM bank
                boundary (512 f32 cols)."""
                pts = sorted({c0, c1} | ({512} if c0 < 512 < c1 else set()))
                for a, b in zip(pts, pts[1:]):
                    nc.tensor.matmul(ps[:, a:b], lhsT, rhs[:, a:b],
                                     start=start, stop=stop)

            # ---------------- shared LN helper ----------------
            def layernorm_stats(xt, ps_stat, W, want_mu_bc, xsq_dve=0,
                                tags=("s1", "s2")):
                """xt: accessor c -> [128, W] AP.  xsq on ACT Square (same
                table set as Ln/Exp) except the first `xsq_dve` tiles on
                DVE for balance."""
                s1 = ps_stat.tile([128, W], f32, tag=tags[0], name="s1")
                s2 = ps_stat.tile([128, W], f32, tag=tags[1], name="s2")
                for c in range(NCT):
                    xsq = wpool.tile([128, W], bf16, tag=f"xsq{c % 2}",
                                     name="xsq", bufs=1)
                    if c < xsq_dve:
                        nc.vector.tensor_tensor(xsq[:], xt(c), xt(c),
                                                alu.mult)
                    else:
                        nc.scalar.square(xsq[:], xt(c))
                    mmF(s1, ones_sb[:], xt(c), 0, W,
                        c == 0, c == NCT - 1)
                    mmF(s2, ones_sb[:], xsq, 0, W,
                        c == 0, c == NCT - 1)
                t_mu = spool.tile([128, W], f32, tag="t_mu", name="t_mu")
                t_m2 = spool.tile([128, W], f32, tag="t_m2", name="t_m2")
                t_v = spool.tile([128, W], f32, tag="t_mu", name="t_v")
                t_ln = spool.tile([128, W], f32, tag="t_m2", name="t_ln")
                nc.vector.tensor_scalar(t_mu[:], s1[:], 1.0 / C, None,
                                        alu.mult)
                nc.vector.tensor_tensor(t_m2[:], t_mu[:], t_mu[:], alu.mult)
                nc.vector.scalar_tensor_tensor(t_v[:], s2[:], 1.0 / C,
                                               t_m2[:], alu.mult,
                                               alu.subtract)
                nc.scalar.activation(t_ln[:], t_v[:], AF.Ln, bias=EPS)
                b1 = spool.tile([128, W], bf16, tag="b1", name="b1", bufs=2)
                nc.scalar.activation(b1[:], t_ln[:], AF.Exp, scale=-0.5)
                corr = spool.tile([2, W], bf16, tag="corr", name="corr",
                                  bufs=2)
                mu_bf = spool.tile([1, W], bf16, tag="mu_bf", name="mu_bf")
                nc.vector.tensor_scalar(mu_bf[:], s1[0:1, :], 1.0 / C, None,
                                        alu.mult)
                nc.vector.memset(corr[0:2, :], 1.0)
                nc.vector.tensor_tensor(corr[0:1, :], mu_bf[:], b1[0:1, :],
                                        alu.mult)
                mu_bc = None
                if want_mu_bc:
                    mu_bc = spool.tile([128, W], bf16, tag="mu_bc",
                                       name="mu_bc")
                    nc.vector.tensor_scalar(mu_bc[:], s1[:], 1.0 / C, None,
                                            alu.mult)
                return b1, corr, mu_bc

            # persistent small inputs (merged DMAs)
            WQKV = 2 * DP + DH
            wqkv_sb = cpool.tile([128, NCT * WQKV], bf16, tag="wqkv",
                                 name="wqkv")
            nc.sync.dma_start(wqkv_sb[:], wqkv)
            wq_sb = [wqkv_sb[:, c * WQKV:c * WQKV + DP] for c in range(NCT)]
            wk_sb = [wqkv_sb[:, c * WQKV + DP:c * WQKV + 2 * DP]
                     for c in range(NCT)]
            wv_sb = [wqkv_sb[:, c * WQKV + 2 * DP:(c + 1) * WQKV]
                     for c in range(NCT)]
            cq_sb = cpool.tile([2, DP], bf16, tag="cq", name="cq")
            ck_sb = cpool.tile([2, DP], bf16, tag="ck", name="ck")
            cv_sb = cpool.tile([2, DH], bf16, tag="cv", name="cv")
            nc.sync.dma_start(cq_sb[:], cq)
            nc.sync.dma_start(ck_sb[:], ck)
            nc.sync.dma_start(cv_sb[:], cv)
            ctx_sb = cpool.tile([128, NCT * CTX], bf16, tag="ctx", name="ctx")
            nc.sync.dma_start(ctx_sb[:], ctx_bf)
            ctxa = lambda c: ctx_sb[:, c * CTX:(c + 1) * CTX]
            cqc_sb = cpool.tile([2, C], bf16, tag="cqc", name="cqc")
            nc.sync.dma_start(cqc_sb[:], cqc)
            xo_sb = [rpool.tile([128, TG], f32, tag=f"xo{c}", name=f"xo{c}")
                     for c in range(NCT)]
            for c in range(NCT):
                nc.sync.dma_start(xo_sb[c][:], x_own[c * 128:(c + 1) * 128, :])

            x2 = [rpool.tile([128, TG], f32, tag=f"x2_{c}", name=f"x2_{c}")
                  for c in range(NCT)]
            x3 = [rpool.tile([128, TG], f32, tag=f"xo{c}", name=f"x3_{c}")
                  for c in range(NCT)]

            a2a_in = dpool.tile([H * DH, TG], bf16, name="a2a_in")
            a2a_out = dpool.tile([H * DH, TG], bf16, name="a2a_out")

            # ============== Phases A+B (pool scoped, freed after) ==========
            with tc.tile_pool(name="attn", bufs=1) as apool:
                cos_sb = [apool.tile([DP, MG], bf16, tag=f"cos{g}",
                                     name=f"cos{g}") for g in range(NMG)]
                sin_sb = [apool.tile([DP, MG], bf16, tag=f"sin{g}",
                                     name=f"sin{g}") for g in range(NMG)]
                # x is host-pre-tiled [128, NMG*NCT*MG]: one DMA per (b,g)
                xbf = [[apool.tile([128, NCT * MG], bf16, tag="xbf",
                                   name="xbf", bufs=2) for _ in range(NMG)]
                       for _ in range(B)]
                nc.sync.dma_start(xbf[0][0][:], x_bf[0][:, 0:NCT * MG])
                for g in range(NMG):
                    nc.sync.dma_start(cos_sb[g][:],
                                      cosT[:, g * MG:(g + 1) * MG])
                    nc.sync.dma_start(sin_sb[g][:],
                                      sinT[:, g * MG:(g + 1) * MG])
                nc.sync.dma_start(xbf[0][1][:],
                                  x_bf[0][:, NCT * MG:2 * NCT * MG])
                # x(1,0)/x(1,1) are issued inside the phase-A loop so their
                # buffer-reuse waits don't head-of-line-block the Sync queue

                q_sb = [[apool.tile([DP, MG], bf16, tag=f"q{bb}_{g}",
                                    name=f"q{bb}_{g}") for g in range(NMG)]
                        for bb in range(B)]
                k_sb = [[apool.tile([DP, MG], bf16, tag=f"k{bb}_{g}",
                                    name=f"k{bb}_{g}") for g in range(NMG)]
                        for bb in range(B)]
                v_sb = [[apool.tile([128, DH + 1], bf16, tag=f"v{bb}_{t}",
                                    name=f"v{bb}_{t}") for t in range(NKT)]
                        for bb in range(B)]
                o_sb = [[apool.tile([DH, MG], bf16, tag=f"o{bb}_{g}",
                                    name=f"o{bb}_{g}") for g in range(NMG)]
                        for bb in range(B)]

                # ---- Phase A: LN1 + QKV + RoPE ----
                with (
                    tc.tile_pool(name="ps_statA", bufs=1,
                                 space="PSUM") as ps_sA,
                    tc.tile_pool(name="ps_projA", bufs=2,
                                 space="PSUM") as ps_pA,
                ):
                    for bb in range(B):
                        for g in range(NMG):
                            if (bb, g) in ((0, 1), (1, 0)):
                                nb, ng = (1, 0) if g == 1 else (1, 1)
                                nc.sync.dma_start(
                                    xbf[nb][ng][:],
                                    x_bf[nb][:, ng * NCT * MG:
                                             (ng + 1) * NCT * MG])
                            xt = xbf[bb][g]
                            xa = lambda c: xt[:, c * MG:(c + 1) * MG]
                            b1, corr, _ = layernorm_stats(xa, ps_sA, MG,
                                                          False, xsq_dve=2)
                            z = [apool.tile([128, MG], bf16, tag=f"z{c}",
                                            name="z", bufs=2)
                                 for c in range(NCT)]
                            for c in range(NCT):
                                nc.vector.tensor_tensor(z[c][:], xa(c),
                                                        b1[:], alu.mult)
                            for wt, ct, dst in ((wq_sb, cq_sb, q_sb),
                                                (wk_sb, ck_sb, k_sb)):
                                ps = ps_pA.tile([DP, MG], f32, tag="proj",
                                                name="ps")
                                for c in range(NCT):
                                    mmF(ps, wt[c][:], z[c], 0, MG,
                                        c == 0, False)
                                mmF(ps, ct[:], corr, 0, MG, False, True)
                                raw = apool.tile([DP, MG], bf16, tag="qkraw",
                                                 name="raw", bufs=2)
                                nc.scalar.copy(raw[:], ps[:])
                                # RoPE: out[d] = raw[d]*cos + raw[d^64]*sin'
                                t1 = apool.tile([DP, MG], bf16, tag="rope1",
                                                name="t1", bufs=2)
                                t2 = apool.tile([DP, MG], bf16, tag="rope2",
                                                name="t2", bufs=2)
                                eng = nc.gpsimd if dst is q_sb else nc.vector
                                nc.vector.tensor_tensor(t1[:], raw[:],
                                                        cos_sb[g][:],
                                                        alu.mult)
                                eng.tensor_tensor(t2[0:64, :],
                                                  raw[64:128, :],
                                                  sin_sb[g][64:128, :],
                                                  alu.mult)
                                eng.tensor_tensor(t2[64:128, :],
                                                  raw[0:64, :],
                                                  sin_sb[g][0:64, :],
                                                  alu.mult)
                                nc.vector.tensor_tensor(dst[bb][g][:], t1[:],
                                                        t2[:], alu.add)
                            for tt in range(KPG):
                                kt = g * KPG + tt
                                ps = ps_pA.tile([128, DH], f32, tag="proj",
                                                name="ps")
                                for c in range(NCT):
                                    nc.tensor.matmul(
                                        ps[:],
                                        z[c][:, tt * 128:(tt + 1) * 128],
                                        wv_sb[c][:], start=(c == 0),
                                        stop=False)
                                nc.tensor.matmul(
                                    ps[:], corr[:, tt * 128:(tt + 1) * 128],
                                    cv_sb[:], start=False, stop=True)
                                nc.scalar.copy(v_sb[bb][kt][:, 0:DH],
                                               ps[:])
                                nc.vector.memset(v_sb[bb][kt][:, DH:DH + 1],
                                                 1.0)

                # ---- Phase B: causal attention ----
                with tc.tile_pool(name="ps_attn", bufs=1,
                                  space="PSUM") as ps_at:
                    with nc.allow_low_precision(
                            reason="softmax reciprocal bf16"):
                        for bb in range(B):
                            for qg in range(NMG):
                                o_ps = ps_at.tile([DH + 1, MG], f32, tag="av",
                                                  name="o_ps", bufs=1)
                                nkt = KPG * qg + KPG
                                for kt in range(nkt):
                                    p = kt - KPG * qg
                                    q0 = 0 if p < 0 else p * 128
                                    s_ps = ps_at.tile([128, MG], f32,
                                                      tag="scores",
                                                      name="s_ps", bufs=2)
                                    mmF(s_ps,
                                        k_sb[bb][kt // KPG][
                                            :, (kt % KPG) * 128:
                                            (kt % KPG + 1) * 128],
                                        q_sb[bb][qg], q0, MG, True, True)
                                    pt = apool.tile([128, MG], bf16,
                                                    tag="ptB", name="pt",
                                                    bufs=2)
                                    nc.scalar.activation(pt[:, q0:MG],
                                                         s_ps[:, q0:MG],
                                                         AF.Exp)
                                    if p >= 0:
                                        nc.vector.tensor_tensor(
                                            pt[:, q0:q0 + 128],
                                            pt[:, q0:q0 + 128],
                                            tri_sb[:], alu.mult)
                                    mmF(o_ps, v_sb[bb][kt][:], pt, q0, MG,
                                        kt == 0, kt == nkt - 1)
                                den = spool.tile([1, MG], f32, tag="den",
                                                 name="den", bufs=2)
                                nc.vector.tensor_copy(den[:],
                                                      o_ps[DH:DH + 1, :])
                                rcp = spool.tile([1, MG], f32, tag="rcp",
                                                 name="rcp", bufs=2)
                                nc.vector.reciprocal_approx_fast(rcp[:],
                                                                 den[:])
                                rcpb = spool.tile([1, MG], bf16, tag="rcpb",
                                                  name="rcpb", bufs=2)
                                nc.vector.tensor_copy(rcpb[:], rcp[:])
                                b_ps = ps_at.tile([128, MG], f32, tag="bcast",
                                                  name="b_ps", bufs=1)
                                mmF(b_ps, ones_sb[0:1, :], rcpb, 0, MG,
                                    True, True)
                                b_sb = apool.tile([128, MG], bf16, tag="bsb",
                                                  name="b_sb", bufs=2)
                                nc.vector.tensor_copy(b_sb[:], b_ps[:])
                                nc.vector.tensor_tensor(o_sb[bb][qg][:],
                                                        o_ps[0:DH, :],
                                                        b_sb[0:DH, :],
                                                        alu.mult)
                                for half in range(2):
                                    j = bb * 4 + qg * 2 + half
                                    nc.sync.dma_start(
                                        a2a_in[j * DH:(j + 1) * DH, :],
                                        o_sb[bb][qg][:, half * TG:
                                                     (half + 1) * TG])

            # fire the exchange; fill the wait with ctx-side cross-attn work
            if SKIP_COLLECTIVE:
                nc.sync.dma_start(a2a_out[:], a2a_in[:])
            else:
                nc.gpsimd.collective_compute(
                    "AllToAll", alu.bypass, replica_groups=RG8,
                    ins=[a2a_in.opt()], outs=[a2a_out.opt()])
            if debug:
                nc.sync.dma_start(dbg_ai, a2a_in[:])
                nc.sync.dma_start(dbg_ao, a2a_out[:])

            with (
                tc.tile_pool(name="cross", bufs=1) as xpool,
                tc.tile_pool(name="wstr", bufs=1) as wstr,
                tc.tile_pool(name="ps_projD", bufs=2, space="PSUM") as ps_pD,
                tc.tile_pool(name="ps_attn2", bufs=1, space="PSUM") as ps_at2,
            ):
                # ---- context-side projections (independent of the A2A) ----
                wkc_sb = xpool.tile([128, NCT * C], bf16, tag="wkc",
                                    name="wkc")
                wvc_sb = xpool.tile([128, NCT * C], bf16, tag="wvc",
                                    name="wvc")
                nc.sync.dma_start(wkc_sb[:], wkc)
                nc.sync.dma_start(wvc_sb[:], wvc)
                kc_sb = [xpool.tile([DH, CTX], bf16, tag=f"kc{h}",
                                    name=f"kc{h}") for h in range(H)]
                for h in range(H):
                    ps = ps_pD.tile([DH, CTX], f32, tag="proj", name="ps")
                    for c in range(NCT):
                        nc.tensor.matmul(
                            ps[:],
                            wkc_sb[:, c * C + h * DH:c * C + (h + 1) * DH],
                            ctxa(c), start=(c == 0), stop=(c == NCT - 1))
                    nc.vector.tensor_copy(kc_sb[h][:], ps[:])
                vc_sb = xpool.tile([128, H * (DH + 1)], bf16, tag="vc",
                                   name="vc")
                for half in range(2):
                    ps = ps_pD.tile([128, C // 2], f32, tag="proj", name="ps")
                    for c in range(NCT):
                        nc.tensor.matmul(
                            ps[:], ctxa(c),
                            wvc_sb[:, c * C + half * 384:
                                   c * C + (half + 1) * 384],
                            start=(c == 0), stop=(c == NCT - 1))
                    dv = vc_sb[:].rearrange("p (h d) -> p h d", h=H)[
                        :, half * 4:(half + 1) * 4, 0:DH]
                    sv = ps[:].rearrange("p (h d) -> p h d", h=4)
                    nc.vector.tensor_copy(dv, sv)
                nc.vector.memset(
                    vc_sb[:].rearrange("p (h d) -> p h d",
                                       h=H)[:, :, DH:DH + 1], 1.0)

                # ---- after the A2A: gather heads, out-project, residual ----
                oa = [xpool.tile([128, TG], bf16, tag=f"oa{k}",
                                 name=f"oa{k}") for k in range(NCT)]
                for k in range(NCT):
                    nc.sync.dma_start(oa[k][:],
                                      a2a_out[k * 128:(k + 1) * 128, :])
                x2bf = [xpool.tile([128, TG], bf16, tag=f"x2bf{c}",
                                   name=f"x2bf{c}") for c in range(NCT)]
                for ot in range(NCT):
                    wot = wstr.tile([128, NCT * 128], bf16, tag="wot",
                                    name="wot", bufs=3)
                    nc.sync.dma_start(wot[:],
                                      wo_t[:, ot * C:(ot + 1) * C])
                    ps = ps_pD.tile([128, TG], f32, tag="proj", name="ps")
                    for k in range(NCT):
                        nc.tensor.matmul(ps[:],
                                         wot[:, k * 128:(k + 1) * 128],
                                         oa[k][:], start=(k == 0),
                                         stop=(k == NCT - 1))
                    nc.vector.tensor_tensor(x2[ot][:], ps[:], xo_sb[ot][:],
                                            alu.add)
                    nc.scalar.copy(x2bf[ot][:], x2[ot][:])
                    if debug:
                        nc.sync.dma_start(dbg_x2[ot * 128:(ot + 1) * 128, :],
                                          x2[ot][:])

                # ---- LN2 + cross-attention ----
                b1, corr, _ = layernorm_stats(lambda c: x2bf[c][:], ps_pD,
                                              TG, False, tags=("proj", "proj"))
                z2 = [xpool.tile([128, TG], bf16, tag=f"z2_{c}",
                                 name=f"z2_{c}") for c in range(NCT)]
                for c in range(NCT):
                    nc.vector.tensor_tensor(z2[c][:], x2bf[c][:], b1[:],
                                            alu.mult)
                ocfm = [xpool.tile([128, TG], bf16, tag=f"oa{c}",
                                   name=f"ocfm{c}") for c in range(NCT)]
                wqct = [wstr.tile([128, NCT * DH], bf16, tag="wqct",
                                  name="wqct", bufs=4) for h in range(H)]
                for h in range(H):
                    nc.sync.dma_start(
                        wqct[h][:],
                        wqc_t[:, h * NCT * DH:(h + 1) * NCT * DH])
                with nc.allow_low_precision(reason="softmax reciprocal bf16"):
                    for h in range(H):
                        qc_ps = ps_pD.tile([DH, TG], f32, tag="proj",
                                           name="ps")
                        for c in range(NCT):
                            nc.tensor.matmul(
                                qc_ps[:], wqct[h][:, c * DH:(c + 1) * DH],
                                z2[c][:], start=(c == 0), stop=False)
                        nc.tensor.matmul(qc_ps[:],
                                         cqc_sb[:, h * DH:(h + 1) * DH],
                                         corr[:], start=False, stop=True)
                        qc = wpool.tile([DH, TG], bf16, tag="qc", name="qc")
                        nc.vector.tensor_copy(qc[:], qc_ps[:])
                        s_ps = ps_at2.tile([CTX, TG], f32, tag="scores",
                                           name="s_ps", bufs=2)
                        nc.tensor.matmul(s_ps[:], kc_sb[h][:], qc[:],
                                         start=True, stop=True)
                        pt = wpool.tile([CTX, TG], bf16, tag="ptD",
                                        name="pt")
                        nc.scalar.activation(pt[:], s_ps[:], AF.Exp)
                        o_ps = ps_at2.tile([DH + 1, TG], f32, tag="av",
                                           name="o_ps", bufs=2)
                        nc.tensor.matmul(
                            o_ps[:],
                            vc_sb[:, h * (DH + 1):(h + 1) * (DH + 1)],
                            pt[:], start=True, stop=True)
                        den = spool.tile([1, TG], f32, tag="den", name="den",
                                         bufs=2)
                        nc.scalar.copy(den[:], o_ps[DH:DH + 1, :])
                        rcp = spool.tile([1, TG], f32, tag="rcp", name="rcp",
                                         bufs=2)
                        nc.vector.reciprocal_approx_fast(rcp[:], den[:])
                        rcpb = spool.tile([1, TG], bf16, tag="rcpb",
                                          name="rcpb", bufs=2)
                        nc.vector.tensor_copy(rcpb[:], rcp[:])
                        b_ps = ps_at2.tile([128, TG], f32, tag="bcast",
                                           name="b_ps", bufs=1)
                        nc.tensor.matmul(b_ps[:], ones_sb[0:1, :], rcpb[:],
                                         start=True, stop=True)
                        b_sb = wpool.tile([128, TG], bf16, tag="bsbD",
                                          name="b_sb")
                        nc.scalar.copy(b_sb[:], b_ps[:])

                        def _maxn(v):
                            if v % 128 == 0:
                                return 128
                            if v % 64 == 0:
                                return 64
                            return 32
                        pos = 0
                        while pos < DH:
                            r = h * DH + pos
                            c0, off = r // 128, r % 128
                            n = min(_maxn(off), _maxn(pos), DH - pos,
                                    128 - off)
                            nc.vector.tensor_tensor(
                                ocfm[c0][off:off + n, :],
                                o_ps[pos:pos + n, :],
                                b_sb[pos:pos + n, :], alu.mult)
                            pos += n

                x3bf = [xpool.tile([128, TG], bf16, tag=f"z2_{c}",
                                   name=f"x3bf{c}") for c in range(NCT)]
                for ot in range(NCT):
                    wcot = wstr.tile([128, NCT * 128], bf16, tag="wcot",
                                     name="wcot", bufs=3)
                    nc.sync.dma_start(wcot[:],
                                      wco_t[:, ot * C:(ot + 1) * C])
                    ps = ps_pD.tile([128, TG], f32, tag="proj", name="ps")
                    for c in range(NCT):
                        nc.tensor.matmul(ps[:],
                                         wcot[:, c * 128:(c + 1) * 128],
                                         ocfm[c][:], start=(c == 0),
                                         stop=(c == NCT - 1))
                    nc.vector.tensor_tensor(x3[ot][:], ps[:], x2[ot][:],
                                            alu.add)
                    nc.scalar.copy(x3bf[ot][:], x3[ot][:])
                    if debug:
                        nc.sync.dma_start(dbg_x3[ot * 128:(ot + 1) * 128, :],
                                          x3[ot][:])

                # ---- LN3 (z3 mean-subtracted on DVE, fp8 DoubleRow pack) --
                b1, corr, mu_bc = layernorm_stats(lambda c: x3bf[c][:],
                                                  ps_pD, TG, True,
                                                  tags=("proj", "proj"))
                z3p = [xpool.tile([128, 2 * TG], fp8, tag=f"z3p{cp}",
                                  name=f"z3p{cp}") for cp in range(3)]
                for c in range(NCT):
                    zt = wpool.tile([128, TG], bf16, tag="zt", name="zt")
                    nc.vector.tensor_tensor(zt[:], x3bf[c][:], mu_bc[:],
                                            alu.subtract)
                    dst = z3p[c // 2][:, (c % 2) * TG:(c % 2 + 1) * TG]
                    nc.vector.tensor_tensor(dst, zt[:], b1[:], alu.mult)
                if not bias_zero:
                    onerow = xpool.tile([1, TG], bf16, tag="onerow",
                                        name="onerow")
                    nc.vector.memset(onerow[:], 1.0)
                    cg_sb = xpool.tile([1, II], bf16, tag="cg", name="cg")
                    cu_sb = xpool.tile([1, II], bf16, tag="cu", name="cu")
                    nc.sync.dma_start(cg_sb[:], cg)
                    nc.sync.dma_start(cu_sb[:], cu)

                # ---- Phase E: SwiGLU FFN, fp8 DoubleRow (weights x64) ----
                hh = [xpool.tile([128, 2 * TG], fp8, tag=f"hh{ip}",
                                 name=f"hh{ip}") for ip in range(12)]
                DR = mybir.MatmulPerfMode.DoubleRow
                if True:
                    for it in range(NIT):
                        wgt = wstr.tile([128, 3 * 256], fp8, tag="wgt",
                                        name="wgt", bufs=3)
                        wut = wstr.tile([128, 3 * 256], fp8, tag="wut",
                                        name="wut", bufs=3)
                        nc.sync.dma_start(
                            wgt[:], wg_t[:, it * 768:(it + 1) * 768])
                        nc.sync.dma_start(
                            wut[:], wu_t[:, it * 768:(it + 1) * 768])
                        g_ps = ps_at2.tile([128, TG], f32, tag="scores",
                                           name="g_ps", bufs=2)
                        u_ps = ps_pD.tile([128, TG], f32, tag="proj",
                                          name="u_ps")
                        for cp in range(3):
                            last = (cp == 2) and bias_zero
                            zr = z3p[cp][:].rearrange("p (r t) -> p r t",
                                                      r=2)
                            for w_, ps_ in ((wgt, g_ps), (wut, u_ps)):
                                wr = w_[:, cp * 256:(cp + 1) * 256].rearrange(
                                    "p (r m) -> p r m", r=2)
                                nc.tensor.matmul(ps_[:], wr, zr,
                                                 start=(cp == 0), stop=last,
                                                 perf_mode=DR)
                        if not bias_zero:
                            nc.tensor.matmul(
                                g_ps[:], cg_sb[:, it * 128:(it + 1) * 128],
                                onerow[:], start=False, stop=True)
                            nc.tensor.matmul(
                                u_ps[:], cu_sb[:, it * 128:(it + 1) * 128],
                                onerow[:], start=False, stop=True)
                        # silu(g_true)*u_true: ACT Silu descales g (x1/64),
                        # the u descale (1/64) folds into the hh write
                        sg = wpool.tile([128, TG], bf16, tag="sg", name="sg")
                        nc.scalar.activation(sg[:], g_ps[:], AF.Silu,
                                             scale=1.0 / 64)
                        hdst = hh[it // 2][:, (it % 2) * TG:
                                           (it % 2 + 1) * TG]
                        nc.vector.scalar_tensor_tensor(hdst, sg[:],
                                                       1.0 / 64, u_ps[:],
                                                       alu.mult, alu.mult)
                if True:
                    for ot in range(NCT):
                        wdt = wstr.tile([128, 12 * 256], fp8, tag="wdt",
                                        name="wdt", bufs=2)
                        nc.sync.dma_start(
                            wdt[:], wd_t[:, ot * 12 * 256:
                                         (ot + 1) * 12 * 256])
                        d_ps = ps_at2.tile([128, TG], f32, tag="scores",
                                           name="d_ps", bufs=2)
                        for ip in range(12):
                            wr = wdt[:, ip * 256:(ip + 1) * 256].rearrange(
                                "p (r m) -> p r m", r=2)
                            hr = hh[ip][:].rearrange("p (r t) -> p r t", r=2)
                            nc.tensor.matmul(d_ps[:], wr, hr,
                                             start=(ip == 0), stop=(ip == 11),
                                             perf_mode=DR)
                        xf = wpool.tile([128, TG], f32, tag="xf", name="xf")
                        nc.vector.scalar_tensor_tensor(xf[:], d_ps[:],
                                                       1.0 / 64, x3[ot][:],
                                                       alu.mult, alu.add)
                        nc.sync.dma_start(out_x[ot * 128:(ot + 1) * 128, :],
                                          xf[:])

    nc.compile()
    return nc


def _rope_tables(head_dim, height, width, frames, base=10000.0):
    d = head_dim // 3
    dx, dy, dt_ = d, d, head_dim - 2 * d

    def freqs(n, dd):
        inv = 1.0 / base ** (np.arange(0, dd, 2, dtype=np.float32) / dd)
        f = np.outer(np.arange(n, dtype=np.float32), inv)
        return np.concatenate([f, f], axis=-1)

    fx, fy, ft = freqs(width, dx), freqs(height, dy), freqs(frames, dt_)
    shp = (frames, height, width)
    cx = np.broadcast_to(np.cos(fx)[None, None, :, :], shp + (dx,))
    sx = np.broadcast_to(np.sin(fx)[None, None, :, :], shp + (dx,))
    cy = np.broadcast_to(np.cos(fy)[None, :, None, :], shp + (dy,))
    sy = np.broadcast_to(np.sin(fy)[None, :, None, :], shp + (dy,))
    ct = np.broadcast_to(np.cos(ft)[:, None, None, :], shp + (dt_,))
    st = np.broadcast_to(np.sin(ft)[:, None, None, :], shp + (dt_,))
    cos = np.concatenate([cx, cy, ct], axis=-1).reshape(-1, head_dim)
    sin = np.concatenate([sx, sy, st], axis=-1).reshape(-1, head_dim)
    return cos.astype(np.float32), sin.astype(np.float32)


def _qk_perm():
    """Stored-index -> original head-dim map (-1 = zero pad), length 128.
    Layout [x1(48) pad16 | x2(48) pad16] puts every rotate-half partner at
    stored index s^64."""
    P = np.full(DP, -1, np.int64)
    P[0:48] = np.arange(0, 48)
    P[64:112] = np.arange(48, 96)
    return P


def _tile6(W, nb):
    """[C, nb*128] -> [128, nb*NCT*128] with block (b, c) at cols
    (b*NCT+c)*128."""
    return np.ascontiguousarray(
        W.reshape(NCT, 128, nb, 128).transpose(1, 2, 0, 3).reshape(
            128, nb * NCT * 128))


def _prep_inputs(inputs):
    """Host-side prep.  Returns (bias_zero, in_maps)."""
    f = lambda k: np.asarray(inputs[k], np.float32)
    x, context = f("x"), f("context")
    wqkv, w_attn_out = f("wqkv"), f("w_attn_out")
    ln1_g, ln1_b = f("ln1_g"), f("ln1_b")
    wq_c, wk_c, wv_c, w_cross_out = (f("wq_c"), f("wk_c"), f("wv_c"),
                                     f("w_cross_out"))
    ln2_g, ln2_b = f("ln2_g"), f("ln2_b")
    w_gate, w_up, w_down = f("w_gate"), f("w_up"), f("w_down")
    ln3_g, ln3_b = f("ln3_g"), f("ln3_b")
    height, width, frames = (int(inputs["height"]), int(inputs["width"]),
                             int(inputs["frames"]))

    bias_zero = bool((ln3_b == 0).all())
    sc = DH ** -0.25

    def fold(W, g, b, scale=1.0):
        Wg = g[:, None] * W * scale
        c0 = -Wg.sum(axis=0)
        c1 = b @ W * scale
        return Wg, np.stack([c0, c1]).astype(BF16)

    wqkv_g, cqkv = fold(wqkv, ln1_g, ln1_b)
    wqkv_g[:, :C] *= sc
    wqkv_g[:, C:2 * C] *= sc
    cqkv[:, :2 * C] *= BF16(sc)
    wqc_g, cqc = fold(wq_c, ln2_g, ln2_b, sc)
    wkc_s = (wk_c * sc).astype(BF16)
    # LN3: mean handled on-chip; fold only the gain.
    wg_g = (ln3_g[:, None] * w_gate).astype(BF16)
    wu_g = (ln3_g[:, None] * w_up).astype(BF16)

    cos, sin = _rope_tables(DH, height, width, frames)
    sinp = sin.copy()
    sinp[:, :DH // 2] *= -1.0
    P = _qk_perm()
    valid = P >= 0
    Pc = np.where(valid, P, 0)
    cosP = np.where(valid[None, :], cos[:, Pc], 0.0)
    sinP = np.where(valid[None, :], sinp[:, Pc], 0.0)
    cosT = np.ascontiguousarray(cosP.T).astype(BF16)
    # sin is read at raw's partition base (SB inputs must share it), so
    # pre-swap columns: sin_sb[d] = sinP[d^64], giving
    # out[d] = raw[d]*cosP[d] + raw[d^64]*sin_sb[d^64] = ... + raw[d^64]*sinP[d]
    sinT = np.ascontiguousarray(sinP[:, np.arange(DP) ^ 64].T).astype(BF16)

    def permute_qk(Wh):  # [rows, DH] -> [rows, DP] permuted+padded
        out = np.zeros((Wh.shape[0], DP), Wh.dtype)
        out[:, valid] = Wh[:, Pc[valid]]
        return out

    tri = np.triu(np.ones((128, 128), np.float32)).astype(BF16)
    ones128 = np.ones((128, 128), np.float32).astype(BF16)

    xT = np.ascontiguousarray(x.transpose(0, 2, 1))          # [B, C, S]
    ctxT = np.ascontiguousarray(context.transpose(0, 2, 1))  # [B, C, CTX]

    # pre-tiled streamed weights (shared across cores)
    wqc_tl = np.ascontiguousarray(
        wqc_g.astype(BF16).reshape(NCT, 128, H, DH).transpose(
            1, 2, 0, 3).reshape(128, H * NCT * DH))
    wo_tl = _tile6(w_attn_out.astype(BF16), NCT)
    wco_tl = _tile6(w_cross_out.astype(BF16), NCT)
    FP8 = ml_dtypes.float8_e4m3fn

    def pack_dr(W, nb):
        # [K, nb*128] -> [128, nb*(K/256)*256] fp8 DoubleRow blocks.
        # Slot (p, parity r) holds contraction row kp*256 + r*128 + p,
        # matching how the kernel packs z3/hh pairs on-chip:
        # lhsT[p, ((b*KP + kp)*128 + m)*2 + r] = W[kp*256 + r*128 + p, b*128+m]
        K = W.shape[0]
        KP = K // 256
        t = W.reshape(KP, 2, 128, nb, 128)          # [kp, r, p, b, m]
        t = t.transpose(2, 3, 0, 1, 4)              # [p, b, kp, r, m]
        return np.ascontiguousarray(t.reshape(128, nb * KP * 256)).astype(FP8)

    wg_tl = pack_dr(np.float32(64.0) * wg_g.astype(np.float32), NIT)
    wu_tl = pack_dr(np.float32(64.0) * wu_g.astype(np.float32), NIT)
    wd_tl = pack_dr(np.float32(64.0) * w_down, NCT)

    def xtile(xb):  # [C, S] -> [128, NMG*NCT*MG], block (g, c)
        return np.ascontiguousarray(
            xb.reshape(NCT, 128, NMG, MG).transpose(1, 2, 0, 3).reshape(
                128, NMG * NCT * MG))

    def rowtile(W, w):  # [C, w] -> [128, NCT*w], block c at cols c*w
        return np.ascontiguousarray(
            W.reshape(NCT, 128, w).transpose(1, 0, 2).reshape(128, NCT * w))

    shared = dict(
        cosT=cosT, sinT=sinT, tri=tri, ones_in=ones128,
        x_bf0=xtile(xT[0].astype(BF16)), x_bf1=xtile(xT[1].astype(BF16)),
        wo_t=wo_tl, wqc_t=wqc_tl, wco_t=wco_tl,
        wkc=rowtile(wkc_s, C), wvc=rowtile(wv_c.astype(BF16), C),
        wg_t=wg_tl, wu_t=wu_tl, wd_t=wd_tl,
        cqc=cqc,
    )
    if not bias_zero:
        shared["cg"] = (ln3_b @ w_gate).astype(BF16)[None, :]
        shared["cu"] = (ln3_b @ w_up).astype(BF16)[None, :]
    in_maps = []
    for core in range(NCORES):
        h = core                      # head owned in phases A/B
        b, gq = core // 4, core % 4   # batch/token-group in phases D/E
        m = dict(shared)
        m["x_own"] = np.ascontiguousarray(xT[b][:, gq * TG:(gq + 1) * TG])
        m["ctx_bf"] = rowtile(ctxT[b].astype(BF16), CTX)
        qs = slice(DH * h, DH * (h + 1))
        wq_h = permute_qk(wqkv_g[:, :C][:, qs]).astype(BF16)
        wk_h = permute_qk(wqkv_g[:, C:2 * C][:, qs]).astype(BF16)
        wv_h = wqkv_g[:, 2 * C:][:, qs].astype(BF16)
        # merged [wq|wk|wv] per 128-row tile: [128, NCT*(2*DP+DH)]
        wqkv_h = np.concatenate([wq_h, wk_h, wv_h], axis=1)  # [C, 352]
        m["wqkv_t"] = np.ascontiguousarray(
            wqkv_h.reshape(NCT, 128, 2 * DP + DH).transpose(1, 0, 2).reshape(
                128, NCT * (2 * DP + DH)))
        m["cq"] = permute_qk(cqkv[:, :C][:, qs].astype(np.float32)
                             ).astype(BF16)
        m["ck"] = permute_qk(cqkv[:, C:2 * C][:, qs].astype(np.float32)
                             ).astype(BF16)
        m["cv"] = np.ascontiguousarray(cqkv[:, 2 * C:][:, qs])
        in_maps.append(m)
    return bias_zero, in_maps


def _get_nc(inputs):
    bias_zero, in_maps = _prep_inputs(inputs)
    key = ("nc", bias_zero)
    if key not in _CACHE:
        _CACHE[key] = _build_program(bias_zero)
    return _CACHE[key], in_maps


def kernel(**inputs):
    from concourse import bass_utils
    nc, in_maps = _get_nc(inputs)
    res = bass_utils.run_bass_kernel_spmd(nc, in_maps,
                                          core_ids=list(range(NCORES)))
    out = np.empty((B, C, S), np.float32)
    for core in range(NCORES):
        b, g = core // 4, core % 4
        out[b][:, g * TG:(g + 1) * TG] = res.results[core]["out_x"]
    return np.ascontiguousarray(out.transpose(0, 2, 1))


# revision 76
# speedup vs baseline: 1.0473x; 1.0473x over previous
"""Causal3DTransformerBlock on 8 TRN2 NeuronCores.

Sharding: self-attention is head-parallel with ONE head x BOTH batches per
core (core h owns head h).  A single 8-core AllToAll redistributes the
attention outputs to token-parallel (core j: batch j//4, tokens
(j%4)*512 .. +512); every A2A block is real data and the gathered rows are
head-major on every core, so the out-projection (full C contraction) runs
after the exchange with the unpermuted w_attn_out.  Cross-attention and the
SwiGLU FFN are token-parallel (no further collectives).

Compute dtype bf16 (fp32 PSUM accumulation, fp32 residual stream).
LayerNorm gains fold into weights host-side; mean/bias enter as a K=2
rank-1 correction matmul, except the FFN where the mean is subtracted on
DVE (the bias path compiles only when ln3_b is nonzero).
Softmax: raw exp (1/sqrt(dh) folded into wq/wk), denominator accumulated
by an extra ones-row in V, normalization via DVE fast reciprocal +
ones-matmul broadcast.  The FFN runs in fp8 (e4m3, weights x64) with
DoubleRow matmuls; SiLU on the ACT engine descales the gate.
Large weights (wqc, wo, wco, wg, wu, wd) are host-pre-tiled and streamed
through small double-buffered SBUF tiles.
"""

import sys

sys.path.insert(0, "/opt/trn_rl_repo")

import os

import numpy as np
import ml_dtypes

BF16 = ml_dtypes.bfloat16
SKIP_COLLECTIVE = bool(int(os.environ.get("K_SKIP_COLLECTIVE", "0")))

B, S, C, CTX, II, H, DH = 2, 2048, 768, 128, 3072, 8, 96
NCORES = 8
TG = 512         # tokens per core after the exchange (A2A block width)
MG = 1024        # megagroup width for self-attention phases
NMG = S // MG    # 2
NKT = S // 128   # 16 key tiles
KPG = MG // 128  # 8 key tiles per megagroup
NCT = C // 128   # 6 feature tiles
NIT = II // 128  # 24 FFN intermediate tiles
DP = 128         # stored (permuted+padded) Q/K head dim
EPS = 1e-5
RG8 = [[0, 1, 2, 3, 4, 5, 6, 7]]

_CACHE = {}


def _build_program(bias_zero, debug=False):
    import concourse.bass as bass
    import concourse.tile as tile
    from concourse import bacc, mybir
    from concourse.alu_op_type import AluOpType as alu

    f32 = mybir.dt.float32
    bf16 = mybir.dt.bfloat16
    AF = mybir.ActivationFunctionType

    nc = bacc.Bacc("TRN2", debug=False, num_devices=NCORES)
    DRA = mybir.MatmulPerfMode.DoubleRow

    def din(name, shape, dt=bf16):
        return nc.dram_tensor(name, shape, dt, kind="ExternalInput").ap()

    # x^T pre-tiled [128, (g,c)-blocks of MG cols], both batches
    x_bf = [din(f"x_bf{b}", [128, NMG * NCT * MG]) for b in range(B)]
    x_own = din("x_own", [C, TG], f32)    # own 512-token slice, fp32
    ctx_bf = din("ctx_bf", [128, NCT * CTX])  # own batch ctx^T, pre-tiled
    cosT = din("cosT", [DP, S])
    sinT = din("sinT", [DP, S])           # sign-folded, partner-swapped (^64)
    tri = din("tri", [128, 128])          # triu {0,1} mask: [k,q] valid q>=k
    ones_in = din("ones_in", [128, 128])
    # fp8 DoubleRow-packed QKV weights (x64): blocks (cpair) of 256 cols
    fp8 = mybir.dt.float8e4
    wq_p = din("wq_p", [128, 3 * 2 * DP], fp8)
    wk_p = din("wk_p", [128, 3 * 2 * DP], fp8)
    wv_p = din("wv_p", [128, 3 * 2 * DH], fp8)
    wkc = din("wkc", [128, NCT * C])      # pre-tiled, block c at cols c*C
    wvc = din("wvc", [128, NCT * C])
    # pre-tiled [128, blocks]: see _prep_inputs for the layouts
    wo_t = din("wo_t", [128, NCT * NCT * 128])
    wqc_t = din("wqc_t", [128, H * NCT * DH])
    wco_t = din("wco_t", [128, NCT * NCT * 128])
    # FFN weights in fp8 (x64 scaled), DoubleRow-packed
    wg_t = din("wg_t", [128, NIT * 3 * 256], fp8)
    wu_t = din("wu_t", [128, NIT * 3 * 256], fp8)
    wd_t = din("wd_t", [128, NCT * 12 * 256], fp8)
    cq = din("cq", [2, DP])               # corrections: row0=-colsum(W'), row1=b@W
    ck = din("ck", [2, DP])
    cv = din("cv", [2, DH])
    cqc = din("cqc", [2, C])
    if not bias_zero:
        cg = din("cg", [1, II])           # b@Wg (bias variant only)
        cu = din("cu", [1, II])

    out_x = nc.dram_tensor("out_x", [C, TG], f32, kind="ExternalOutput").ap()
    if debug:
        dbg_ai = nc.dram_tensor("dbg_ai", [H * DH, TG], bf16,
                                kind="ExternalOutput").ap()
        dbg_ao = nc.dram_tensor("dbg_ao", [H * DH, TG], bf16,
                                kind="ExternalOutput").ap()
        dbg_x2 = nc.dram_tensor("dbg_x2", [C, TG], f32,
                                kind="ExternalOutput").ap()
        dbg_x3 = nc.dram_tensor("dbg_x3", [C, TG], f32,
                                kind="ExternalOutput").ap()

    with tile.TileContext(nc) as tc:
        with (
            tc.tile_pool(name="const", bufs=1) as cpool,
            tc.tile_pool(name="resid", bufs=1) as rpool,
            tc.tile_pool(name="work", bufs=2) as wpool,
            tc.tile_pool(name="stat", bufs=1) as spool,
            tc.tile_pool(name="dram", bufs=1, space="DRAM") as dpool,
        ):
            # ---- const APs for activation bias ----
            czero = cpool.tile([128, 1], f32, tag="czero", name="czero")
            nc.vector.memset(czero[:], 0.0)
            nc.const_aps.aps[(f32, 0.0)] = czero[:]
            ceps = cpool.tile([128, 1], f32, tag="ceps", name="ceps")
            nc.vector.memset(ceps[:], EPS)
            nc.const_aps.aps[(f32, EPS)] = ceps[:]
            ones_sb = cpool.tile([128, 128], bf16, tag="ones", name="ones")
            nc.sync.dma_start(ones_sb[:], ones_in)
            tri_sb = cpool.tile([128, 128], bf16, tag="tri", name="tri")
            nc.sync.dma_start(tri_sb[:], tri)


            def mmF(ps, lhsT, rhs, c0, c1, start, stop):
                """matmul on cols [c0:c1) of ps/rhs, split at the PSUM bank
                boundary (512 f32 cols)."""
                pts = sorted({c0, c1} | ({512} if c0 < 512 < c1 else set()))
                for a, b in zip(pts, pts[1:]):
                    nc.tensor.matmul(ps[:, a:b], lhsT, rhs[:, a:b],
                                     start=start, stop=stop)

            # ---------------- shared LN helper ----------------
            def layernorm_stats(xt, ps_stat, W, want_mu_bc, xsq_dve=0,
                                tags=("s1", "s2")):
                """xt: accessor c -> [128, W] AP.  xsq on ACT Square (same
                table set as Ln/Exp) except the first `xsq_dve` tiles on
                DVE for balance."""
                s1 = ps_stat.tile([128, W], f32, tag=tags[0], name="s1")
                s2 = ps_stat.tile([128, W], f32, tag=tags[1], name="s2")
                for c in range(NCT):
                    xsq = wpool.tile([128, W], bf16, tag=f"xsq{c % 2}",
                                     name="xsq", bufs=1)
                    if c < xsq_dve:
                        nc.vector.tensor_tensor(xsq[:], xt(c), xt(c),
                                                alu.mult)
                    else:
                        nc.scalar.square(xsq[:], xt(c))
                    mmF(s1, ones_sb[:], xt(c), 0, W,
                        c == 0, c == NCT - 1)
                    mmF(s2, ones_sb[:], xsq, 0, W,
                        c == 0, c == NCT - 1)
                t_mu = spool.tile([128, W], f32, tag="t_mu", name="t_mu")
                t_m2 = spool.tile([128, W], f32, tag="t_m2", name="t_m2")
                t_v = spool.tile([128, W], f32, tag="t_mu", name="t_v")
                t_ln = spool.tile([128, W], f32, tag="t_m2", name="t_ln")
                nc.vector.tensor_scalar(t_mu[:], s1[:], 1.0 / C, None,
                                        alu.mult)
                nc.vector.tensor_tensor(t_m2[:], t_mu[:], t_mu[:], alu.mult)
                nc.vector.scalar_tensor_tensor(t_v[:], s2[:], 1.0 / C,
                                               t_m2[:], alu.mult,
                                               alu.subtract)
                nc.scalar.activation(t_ln[:], t_v[:], AF.Ln, bias=EPS)
                b1 = spool.tile([128, W], bf16, tag="b1", name="b1", bufs=2)
                nc.scalar.activation(b1[:], t_ln[:], AF.Exp, scale=-0.5)
                corr = spool.tile([2, W], bf16, tag="corr", name="corr",
                                  bufs=2)
                mu_bf = spool.tile([1, W], bf16, tag="mu_bf", name="mu_bf")
                nc.vector.tensor_scalar(mu_bf[:], s1[0:1, :], 1.0 / C, None,
                                        alu.mult)
                nc.vector.memset(corr[0:2, :], 1.0)
                nc.vector.tensor_tensor(corr[0:1, :], mu_bf[:], b1[0:1, :],
                                        alu.mult)
                mu_bc = None
                if want_mu_bc:
                    mu_bc = spool.tile([128, W], bf16, tag="mu_bc",
                                       name="mu_bc")
                    nc.vector.tensor_scalar(mu_bc[:], s1[:], 1.0 / C, None,
                                            alu.mult)
                return b1, corr, mu_bc

            # persistent small inputs (fp8 DoubleRow QKV weights)
            wq_sb = cpool.tile([128, 3 * 2 * DP], fp8, tag="wqp", name="wqp")
            wk_sb = cpool.tile([128, 3 * 2 * DP], fp8, tag="wkp", name="wkp")
            wv_sb = cpool.tile([128, 3 * 2 * DH], fp8, tag="wvp", name="wvp")
            nc.sync.dma_start(wq_sb[:], wq_p)
            nc.sync.dma_start(wk_sb[:], wk_p)
            nc.sync.dma_start(wv_sb[:], wv_p)
            cq_sb = cpool.tile([2, DP], bf16, tag="cq", name="cq")
            ck_sb = cpool.tile([2, DP], bf16, tag="ck", name="ck")
            cv_sb = cpool.tile([2, DH], bf16, tag="cv", name="cv")
            nc.sync.dma_start(cq_sb[:], cq)
            nc.sync.dma_start(ck_sb[:], ck)
            nc.sync.dma_start(cv_sb[:], cv)
            ctx_sb = cpool.tile([128, NCT * CTX], bf16, tag="ctx", name="ctx")
            nc.sync.dma_start(ctx_sb[:], ctx_bf)
            ctxa = lambda c: ctx_sb[:, c * CTX:(c + 1) * CTX]
            cqc_sb = cpool.tile([2, C], bf16, tag="cqc", name="cqc")
            nc.sync.dma_start(cqc_sb[:], cqc)
            xo_sb = [rpool.tile([128, TG], f32, tag=f"xo{c}", name=f"xo{c}")
                     for c in range(NCT)]
            for c in range(NCT):
                nc.sync.dma_start(xo_sb[c][:], x_own[c * 128:(c + 1) * 128, :])

            x2 = [rpool.tile([128, TG], f32, tag=f"x2_{c}", name=f"x2_{c}")
                  for c in range(NCT)]
            x3 = [rpool.tile([128, TG], f32, tag=f"xo{c}", name=f"x3_{c}")
                  for c in range(NCT)]

            a2a_in = dpool.tile([H * DH, TG], bf16, name="a2a_in")
            a2a_out = dpool.tile([H * DH, TG], bf16, name="a2a_out")

            # ============== Phases A+B (pool scoped, freed after) ==========
            with tc.tile_pool(name="attn", bufs=1) as apool:
                cos_sb = [apool.tile([DP, MG], bf16, tag=f"cos{g}",
                                     name=f"cos{g}") for g in range(NMG)]
                sin_sb = [apool.tile([DP, MG], bf16, tag=f"sin{g}",
                                     name=f"sin{g}") for g in range(NMG)]
                # x is host-pre-tiled [128, NMG*NCT*MG]: one DMA per (b,g)
                xbf = [[apool.tile([128, NCT * MG], bf16, tag="xbf",
                                   name="xbf", bufs=2) for _ in range(NMG)]
                       for _ in range(B)]
                nc.sync.dma_start(xbf[0][0][:], x_bf[0][:, 0:NCT * MG])
                for g in range(NMG):
                    nc.sync.dma_start(cos_sb[g][:],
                                      cosT[:, g * MG:(g + 1) * MG])
                    nc.sync.dma_start(sin_sb[g][:],
                                      sinT[:, g * MG:(g + 1) * MG])
                nc.sync.dma_start(xbf[0][1][:],
                                  x_bf[0][:, NCT * MG:2 * NCT * MG])
                # x(1,0)/x(1,1) are issued inside the phase-A loop so their
                # buffer-reuse waits don't head-of-line-block the Sync queue

                q_sb = [[apool.tile([DP, MG], bf16, tag=f"q{bb}_{g}",
                                    name=f"q{bb}_{g}") for g in range(NMG)]
                        for bb in range(B)]
                k_sb = [[apool.tile([DP, MG], bf16, tag=f"k{bb}_{g}",
                                    name=f"k{bb}_{g}") for g in range(NMG)]
                        for bb in range(B)]
                v_sb = [[apool.tile([128, DH + 1], bf16, tag=f"v{bb}_{t}",
                                    name=f"v{bb}_{t}") for t in range(NKT)]
                        for bb in range(B)]
                o_sb = [[apool.tile([DH, MG], bf16, tag=f"o{bb}_{g}",
                                    name=f"o{bb}_{g}") for g in range(NMG)]
                        for bb in range(B)]

                # ---- Phase A: LN1 + QKV + RoPE ----
                with (
                    tc.tile_pool(name="ps_statA", bufs=1,
                                 space="PSUM") as ps_sA,
                    tc.tile_pool(name="ps_projA", bufs=2,
                                 space="PSUM") as ps_pA,
                ):
                    for bb in range(B):
                        for g in range(NMG):
                            if (bb, g) in ((0, 1), (1, 0)):
                                nb, ng = (1, 0) if g == 1 else (1, 1)
                                nc.sync.dma_start(
                                    xbf[nb][ng][:],
                                    x_bf[nb][:, ng * NCT * MG:
                                             (ng + 1) * NCT * MG])
                            xt = xbf[bb][g]
                            xa = lambda c: xt[:, c * MG:(c + 1) * MG]
                            b1, corr, _ = layernorm_stats(xa, ps_sA, MG,
                                                          False, xsq_dve=2)
                            zp = [apool.tile([128, 2 * MG], fp8,
                                             tag=f"zp{cp}", name="zp",
                                             bufs=2) for cp in range(3)]
                            for c in range(NCT):
                                nc.vector.tensor_tensor(
                                    zp[c // 2][:, (c % 2) * MG:
                                               (c % 2 + 1) * MG],
                                    xa(c), b1[:], alu.mult)
                            zr = [zp[cp][:].rearrange("p (r t) -> p r t",
                                                      r=2) for cp in range(3)]
                            for wt, ct, dst in ((wq_sb, cq_sb, q_sb),
                                                (wk_sb, ck_sb, k_sb)):
                                ps = ps_pA.tile([DP, MG], f32, tag="proj",
                                                name="ps")
                                for cp in range(3):
                                    wr = wt[:, cp * 256:(cp + 1) * 256]\
                                        .rearrange("p (r m) -> p r m", r=2)
                                    for a, b_ in ((0, 512), (512, MG)):
                                        nc.tensor.matmul(
                                            ps[:, a:b_], wr,
                                            zr[cp][:, :, a:b_],
                                            start=(cp == 0), stop=False,
                                            perf_mode=DRA)
                                mmF(ps, ct[:], corr, 0, MG, False, True)
                                raw = apool.tile([DP, MG], bf16, tag="qkraw",
                                                 name="raw", bufs=2)
                                nc.scalar.copy(raw[:], ps[:])
                                # RoPE: out[d] = raw[d]*cos + raw[d^64]*sin'
                                t1 = apool.tile([DP, MG], bf16, tag="rope1",
                                                name="t1", bufs=2)
                                t2 = apool.tile([DP, MG], bf16, tag="rope2",
                                                name="t2", bufs=2)
                                eng = nc.gpsimd if dst is q_sb else nc.vector
                                nc.vector.tensor_tensor(t1[:], raw[:],
                                                        cos_sb[g][:],
                                                        alu.mult)
                                eng.tensor_tensor(t2[0:64, :],
                                                  raw[64:128, :],
                                                  sin_sb[g][64:128, :],
                                                  alu.mult)
                                eng.tensor_tensor(t2[64:128, :],
                                                  raw[0:64, :],
                                                  sin_sb[g][0:64, :],
                                                  alu.mult)
                                nc.vector.tensor_tensor(dst[bb][g][:], t1[:],
                                                        t2[:], alu.add)
                            wvr = wv_sb[:].rearrange(
                                "p (k r m) -> p k r m", k=3, r=2)
                            for tt in range(KPG):
                                kt = g * KPG + tt
                                ps = ps_pA.tile([128, DH], f32, tag="proj",
                                                name="ps")
                                for cp in range(3):
                                    nc.tensor.matmul(
                                        ps[:],
                                        zr[cp][:, :, tt * 128:(tt + 1) * 128],
                                        wvr[:, cp], start=(cp == 0),
                                        stop=False, perf_mode=DRA)
                                nc.tensor.matmul(
                                    ps[:], corr[:, tt * 128:(tt + 1) * 128],
                                    cv_sb[:], start=False, stop=True)
                                nc.scalar.activation(v_sb[bb][kt][:, 0:DH],
                                                     ps[:], AF.Copy,
                                                     scale=1.0 / 64)
                                nc.vector.memset(v_sb[bb][kt][:, DH:DH + 1],
                                                 1.0)

                # ---- Phase B: causal attention ----
                with tc.tile_pool(name="ps_attn", bufs=1,
                                  space="PSUM") as ps_at:
                    with nc.allow_low_precision(
                            reason="softmax reciprocal bf16"):
                        for bb in range(B):
                            for qg in range(NMG):
                                o_ps = ps_at.tile([DH + 1, MG], f32, tag="av",
                                                  name="o_ps", bufs=1)
                                nkt = KPG * qg + KPG
                                for kt in range(nkt):
                                    p = kt - KPG * qg
                                    q0 = 0 if p < 0 else p * 128
                                    s_ps = ps_at.tile([128, MG], f32,
                                                      tag="scores",
                                                      name="s_ps", bufs=2)
                                    mmF(s_ps,
                                        k_sb[bb][kt // KPG][
                                            :, (kt % KPG) * 128:
                                            (kt % KPG + 1) * 128],
                                        q_sb[bb][qg], q0, MG, True, True)
                                    pt = apool.tile([128, MG], bf16,
                                                    tag="ptB", name="pt",
                                                    bufs=2)
                                    nc.scalar.activation(pt[:, q0:MG],
                                                         s_ps[:, q0:MG],
                                                         AF.Exp)
                                    if p >= 0:
                                        nc.vector.tensor_tensor(
                                            pt[:, q0:q0 + 128],
                                            pt[:, q0:q0 + 128],
                                            tri_sb[:], alu.mult)
                                    mmF(o_ps, v_sb[bb][kt][:], pt, q0, MG,
                                        kt == 0, kt == nkt - 1)
                                den = spool.tile([1, MG], f32, tag="den",
                                                 name="den", bufs=2)
                                nc.vector.tensor_copy(den[:],
                                                      o_ps[DH:DH + 1, :])
                                rcp = spool.tile([1, MG], f32, tag="rcp",
                                                 name="rcp", bufs=2)
                                nc.vector.reciprocal_approx_fast(rcp[:],
                                                                 den[:])
                                rcpb = spool.tile([1, MG], bf16, tag="rcpb",
                                                  name="rcpb", bufs=2)
                                nc.vector.tensor_copy(rcpb[:], rcp[:])
                                b_ps = ps_at.tile([128, MG], f32, tag="bcast",
                                                  name="b_ps", bufs=1)
                                mmF(b_ps, ones_sb[0:1, :], rcpb, 0, MG,
                                    True, True)
                                b_sb = apool.tile([128, MG], bf16, tag="bsb",
                                                  name="b_sb", bufs=2)
                                nc.vector.tensor_copy(b_sb[:], b_ps[:])
                                nc.vector.tensor_tensor(o_sb[bb][qg][:],
                                                        o_ps[0:DH, :],
                                                        b_sb[0:DH, :],
                                                        alu.mult)
                                for half in range(2):
                                    j = bb * 4 + qg * 2 + half
                                    nc.sync.dma_start(
                                        a2a_in[j * DH:(j + 1) * DH, :],
                                        o_sb[bb][qg][:, half * TG:
                                                     (half + 1) * TG])

            # fire the exchange; fill the wait with ctx-side cross-attn work
            if SKIP_COLLECTIVE:
                nc.sync.dma_start(a2a_out[:], a2a_in[:])
            else:
                nc.gpsimd.collective_compute(
                    "AllToAll", alu.bypass, replica_groups=RG8,
                    ins=[a2a_in.opt()], outs=[a2a_out.opt()])
            if debug:
                nc.sync.dma_start(dbg_ai, a2a_in[:])
                nc.sync.dma_start(dbg_ao, a2a_out[:])

            with (
                tc.tile_pool(name="cross", bufs=1) as xpool,
                tc.tile_pool(name="wstr", bufs=1) as wstr,
                tc.tile_pool(name="ps_projD", bufs=2, space="PSUM") as ps_pD,
                tc.tile_pool(name="ps_attn2", bufs=1, space="PSUM") as ps_at2,
            ):
                # ---- context-side projections (independent of the A2A) ----
                wkc_sb = xpool.tile([128, NCT * C], bf16, tag="wkc",
                                    name="wkc")
                wvc_sb = xpool.tile([128, NCT * C], bf16, tag="wvc",
                                    name="wvc")
                nc.sync.dma_start(wkc_sb[:], wkc)
                nc.sync.dma_start(wvc_sb[:], wvc)
                kc_sb = [xpool.tile([DH, CTX], bf16, tag=f"kc{h}",
                                    name=f"kc{h}") for h in range(H)]
                for h in range(H):
                    ps = ps_pD.tile([DH, CTX], f32, tag="proj", name="ps")
                    for c in range(NCT):
                        nc.tensor.matmul(
                            ps[:],
                            wkc_sb[:, c * C + h * DH:c * C + (h + 1) * DH],
                            ctxa(c), start=(c == 0), stop=(c == NCT - 1))
                    nc.vector.tensor_copy(kc_sb[h][:], ps[:])
                vc_sb = xpool.tile([128, H * (DH + 1)], bf16, tag="vc",
                                   name="vc")
                for half in range(2):
                    ps = ps_pD.tile([128, C // 2], f32, tag="proj", name="ps")
                    for c in range(NCT):
                        nc.tensor.matmul(
                            ps[:], ctxa(c),
                            wvc_sb[:, c * C + half * 384:
                                   c * C + (half + 1) * 384],
                            start=(c == 0), stop=(c == NCT - 1))
                    dv = vc_sb[:].rearrange("p (h d) -> p h d", h=H)[
                        :, half * 4:(half + 1) * 4, 0:DH]
                    sv = ps[:].rearrange("p (h d) -> p h d", h=4)
                    nc.vector.tensor_copy(dv, sv)
                nc.vector.memset(
                    vc_sb[:].rearrange("p (h d) -> p h d",
                                       h=H)[:, :, DH:DH + 1], 1.0)

                # ---- after the A2A: gather heads, out-project, residual ----
                oa = [xpool.tile([128, TG], bf16, tag=f"oa{k}",
                                 name=f"oa{k}") for k in range(NCT)]
                for k in range(NCT):
                    nc.sync.dma_start(oa[k][:],
                                      a2a_out[k * 128:(k + 1) * 128, :])
                x2bf = [xpool.tile([128, TG], bf16, tag=f"x2bf{c}",
                                   name=f"x2bf{c}") for c in range(NCT)]
                for ot in range(NCT):
                    wot = wstr.tile([128, NCT * 128], bf16, tag="wot",
                                    name="wot", bufs=3)
                    nc.sync.dma_start(wot[:],
                                      wo_t[:, ot * C:(ot + 1) * C])
                    ps = ps_pD.tile([128, TG], f32, tag="proj", name="ps")
                    for k in range(NCT):
                        nc.tensor.matmul(ps[:],
                                         wot[:, k * 128:(k + 1) * 128],
                                         oa[k][:], start=(k == 0),
                                         stop=(k == NCT - 1))
                    nc.vector.tensor_tensor(x2[ot][:], ps[:], xo_sb[ot][:],
                                            alu.add)
                    nc.scalar.copy(x2bf[ot][:], x2[ot][:])
                    if debug:
                        nc.sync.dma_start(dbg_x2[ot * 128:(ot + 1) * 128, :],
                                          x2[ot][:])

                # ---- LN2 + cross-attention ----
                b1, corr, _ = layernorm_stats(lambda c: x2bf[c][:], ps_pD,
                                              TG, False, tags=("proj", "proj"))
                z2 = [xpool.tile([128, TG], bf16, tag=f"z2_{c}",
                                 name=f"z2_{c}") for c in range(NCT)]
                for c in range(NCT):
                    nc.vector.tensor_tensor(z2[c][:], x2bf[c][:], b1[:],
                                            alu.mult)
                ocfm = [xpool.tile([128, TG], bf16, tag=f"oa{c}",
                                   name=f"ocfm{c}") for c in range(NCT)]
                wqct = [wstr.tile([128, NCT * DH], bf16, tag="wqct",
                                  name="wqct", bufs=4) for h in range(H)]
                for h in range(H):
                    nc.sync.dma_start(
                        wqct[h][:],
                        wqc_t[:, h * NCT * DH:(h + 1) * NCT * DH])
                with nc.allow_low_precision(reason="softmax reciprocal bf16"):
                    for h in range(H):
                        qc_ps = ps_pD.tile([DH, TG], f32, tag="proj",
                                           name="ps")
                        for c in range(NCT):
                            nc.tensor.matmul(
                                qc_ps[:], wqct[h][:, c * DH:(c + 1) * DH],
                                z2[c][:], start=(c == 0), stop=False)
                        nc.tensor.matmul(qc_ps[:],
                                         cqc_sb[:, h * DH:(h + 1) * DH],
                                         corr[:], start=False, stop=True)
                        qc = wpool.tile([DH, TG], bf16, tag="qc", name="qc")
                        nc.vector.tensor_copy(qc[:], qc_ps[:])
                        s_ps = ps_at2.tile([CTX, TG], f32, tag="scores",
                                           name="s_ps", bufs=2)
                        nc.tensor.matmul(s_ps[:], kc_sb[h][:], qc[:],
                                         start=True, stop=True)
                        pt = wpool.tile([CTX, TG], bf16, tag="ptD",
                                        name="pt")
                        nc.scalar.activation(pt[:], s_ps[:], AF.Exp)
                        o_ps = ps_at2.tile([DH + 1, TG], f32, tag="av",
                                           name="o_ps", bufs=2)
                        nc.tensor.matmul(
                            o_ps[:],
                            vc_sb[:, h * (DH + 1):(h + 1) * (DH + 1)],
                            pt[:], start=True, stop=True)
                        den = spool.tile([1, TG], f32, tag="den", name="den",
                                         bufs=2)
                        nc.scalar.copy(den[:], o_ps[DH:DH + 1, :])
                        rcp = spool.tile([1, TG], f32, tag="rcp", name="rcp",
                                         bufs=2)
                        nc.vector.reciprocal_approx_fast(rcp[:], den[:])
                        rcpb = spool.tile([1, TG], bf16, tag="rcpb",
                                          name="rcpb", bufs=2)
                        nc.vector.tensor_copy(rcpb[:], rcp[:])
                        b_ps = ps_at2.tile([128, TG], f32, tag="bcast",
                                           name="b_ps", bufs=1)
                        nc.tensor.matmul(b_ps[:], ones_sb[0:1, :], rcpb[:],
                                         start=True, stop=True)
                        b_sb = wpool.tile([128, TG], bf16, tag="bsbD",
                                          name="b_sb")
                        nc.scalar.copy(b_sb[:], b_ps[:])

                        def _maxn(v):
                            if v % 128 == 0:
                                return 128
                            if v % 64 == 0:
                                return 64
                            return 32
                        pos = 0
                        while pos < DH:
                            r = h * DH + pos
                            c0, off = r // 128, r % 128
                            n = min(_maxn(off), _maxn(pos), DH - pos,
                                    128 - off)
                            nc.vector.tensor_tensor(
                                ocfm[c0][off:off + n, :],
                                o_ps[pos:pos + n, :],
                                b_sb[pos:pos + n, :], alu.mult)
                            pos += n

                x3bf = [xpool.tile([128, TG], bf16, tag=f"z2_{c}",
                                   name=f"x3bf{c}") for c in range(NCT)]
                for ot in range(NCT):
                    wcot = wstr.tile([128, NCT * 128], bf16, tag="wcot",
                                     name="wcot", bufs=3)
                    nc.sync.dma_start(wcot[:],
                                      wco_t[:, ot * C:(ot + 1) * C])
                    ps = ps_pD.tile([128, TG], f32, tag="proj", name="ps")
                    for c in range(NCT):
                        nc.tensor.matmul(ps[:],
                                         wcot[:, c * 128:(c + 1) * 128],
                                         ocfm[c][:], start=(c == 0),
                                         stop=(c == NCT - 1))
                    nc.vector.tensor_tensor(x3[ot][:], ps[:], x2[ot][:],
                                            alu.add)
                    nc.scalar.copy(x3bf[ot][:], x3[ot][:])
                    if debug:
                        nc.sync.dma_start(dbg_x3[ot * 128:(ot + 1) * 128, :],
                                          x3[ot][:])

                # ---- LN3 (z3 mean-subtracted on DVE, fp8 DoubleRow pack) --
                b1, corr, mu_bc = layernorm_stats(lambda c: x3bf[c][:],
                                                  ps_pD, TG, True,
                                                  tags=("proj", "proj"))
                z3p = [xpool.tile([128, 2 * TG], fp8, tag=f"z3p{cp}",
                                  name=f"z3p{cp}") for cp in range(3)]
                for c in range(NCT):
                    zt = wpool.tile([128, TG], bf16, tag="zt", name="zt")
                    nc.vector.tensor_tensor(zt[:], x3bf[c][:], mu_bc[:],
                                            alu.subtract)
                    dst = z3p[c // 2][:, (c % 2) * TG:(c % 2 + 1) * TG]
                    nc.vector.tensor_tensor(dst, zt[:], b1[:], alu.mult)
                if not bias_zero:
                    onerow = xpool.tile([1, TG], bf16, tag="onerow",
                                        name="onerow")
                    nc.vector.memset(onerow[:], 1.0)
                    cg_sb = xpool.tile([1, II], bf16, tag="cg", name="cg")
                    cu_sb = xpool.tile([1, II], bf16, tag="cu", name="cu")
                    nc.sync.dma_start(cg_sb[:], cg)
                    nc.sync.dma_start(cu_sb[:], cu)

                # ---- Phase E: SwiGLU FFN, fp8 DoubleRow (weights x64) ----
                hh = [xpool.tile([128, 2 * TG], fp8, tag=f"hh{ip}",
                                 name=f"hh{ip}") for ip in range(12)]
                DR = mybir.MatmulPerfMode.DoubleRow
                if True:
                    for it in range(NIT):
                        wgt = wstr.tile([128, 3 * 256], fp8, tag="wgt",
                                        name="wgt", bufs=3)
                        wut = wstr.tile([128, 3 * 256], fp8, tag="wut",
                                        name="wut", bufs=3)
                        nc.sync.dma_start(
                            wgt[:], wg_t[:, it * 768:(it + 1) * 768])
                        nc.sync.dma_start(
                            wut[:], wu_t[:, it * 768:(it + 1) * 768])
                        g_ps = ps_at2.tile([128, TG], f32, tag="scores",
                                           name="g_ps", bufs=2)
                        u_ps = ps_pD.tile([128, TG], f32, tag="proj",
                                          name="u_ps")
                        for cp in range(3):
                            last = (cp == 2) and bias_zero
                            zr = z3p[cp][:].rearrange("p (r t) -> p r t",
                                                      r=2)
                            for w_, ps_ in ((wgt, g_ps), (wut, u_ps)):
                                wr = w_[:, cp * 256:(cp + 1) * 256].rearrange(
                                    "p (r m) -> p r m", r=2)
                                nc.tensor.matmul(ps_[:], wr, zr,
                                                 start=(cp == 0), stop=last,
                                                 perf_mode=DR)
                        if not bias_zero:
                            nc.tensor.matmul(
                                g_ps[:], cg_sb[:, it * 128:(it + 1) * 128],
                                onerow[:], start=False, stop=True)
                            nc.tensor.matmul(
                                u_ps[:], cu_sb[:, it * 128:(it + 1) * 128],
                                onerow[:], start=False, stop=True)
                        # silu(g_true)*u_true: ACT Silu descales g (x1/64),
                        # the u descale (1/64) folds into the hh write
                        sg = wpool.tile([128, TG], bf16, tag="sg", name="sg")
                        nc.scalar.activation(sg[:], g_ps[:], AF.Silu,
                                             scale=1.0 / 64)
                        hdst = hh[it // 2][:, (it % 2) * TG:
                                           (it % 2 + 1) * TG]
                        nc.vector.scalar_tensor_tensor(hdst, sg[:],
                                                       1.0 / 64, u_ps[:],
                                                       alu.mult, alu.mult)
                if True:
                    for ot in range(NCT):
                        wdt = wstr.tile([128, 12 * 256], fp8, tag="wdt",
                                        name="wdt", bufs=2)
                        nc.sync.dma_start(
                            wdt[:], wd_t[:, ot * 12 * 256:
                                         (ot + 1) * 12 * 256])
                        d_ps = ps_at2.tile([128, TG], f32, tag="scores",
                                           name="d_ps", bufs=2)
                        for ip in range(12):
                            wr = wdt[:, ip * 256:(ip + 1) * 256].rearrange(
                                "p (r m) -> p r m", r=2)
                            hr = hh[ip][:].rearrange("p (r t) -> p r t", r=2)
                            nc.tensor.matmul(d_ps[:], wr, hr,
                                             start=(ip == 0), stop=(ip == 11),
                                             perf_mode=DR)
                        xf = wpool.tile([128, TG], f32, tag="xf", name="xf")
                        nc.vector.scalar_tensor_tensor(xf[:], d_ps[:],
                                                       1.0 / 64, x3[ot][:],
                                                       alu.mult, alu.add)
                        nc.sync.dma_start(out_x[ot * 128:(ot + 1) * 128, :],
                                          xf[:])

    nc.compile()
    return nc


def _rope_tables(head_dim, height, width, frames, base=10000.0):
    d = head_dim // 3
    dx, dy, dt_ = d, d, head_dim - 2 * d

    def freqs(n, dd):
        inv = 1.0 / base ** (np.arange(0, dd, 2, dtype=np.float32) / dd)
        f = np.outer(np.arange(n, dtype=np.float32), inv)
        return np.concatenate([f, f], axis=-1)

    fx, fy, ft = freqs(width, dx), freqs(height, dy), freqs(frames, dt_)
    shp = (frames, height, width)
    cx = np.broadcast_to(np.cos(fx)[None, None, :, :], shp + (dx,))
    sx = np.broadcast_to(np.sin(fx)[None, None, :, :], shp + (dx,))
    cy = np.broadcast_to(np.cos(fy)[None, :, None, :], shp + (dy,))
    sy = np.broadcast_to(np.sin(fy)[None, :, None, :], shp + (dy,))
    ct = np.broadcast_to(np.cos(ft)[:, None, None, :], shp + (dt_,))
    st = np.broadcast_to(np.sin(ft)[:, None, None, :], shp + (dt_,))
    cos = np.concatenate([cx, cy, ct], axis=-1).reshape(-1, head_dim)
    sin = np.concatenate([sx, sy, st], axis=-1).reshape(-1, head_dim)
    return cos.astype(np.float32), sin.astype(np.float32)


def _qk_perm():
    """Stored-index -> original head-dim map (-1 = zero pad), length 128.
    Layout [x1(48) pad16 | x2(48) pad16] puts every rotate-half partner at
    stored index s^64."""
    P = np.full(DP, -1, np.int64)
    P[0:48] = np.arange(0, 48)
    P[64:112] = np.arange(48, 96)
    return P


def _tile6(W, nb):
    """[C, nb*128] -> [128, nb*NCT*128] with block (b, c) at cols
    (b*NCT+c)*128."""
    return np.ascontiguousarray(
        W.reshape(NCT, 128, nb, 128).transpose(1, 2, 0, 3).reshape(
            128, nb * NCT * 128))


def _prep_inputs(inputs):
    """Host-side prep.  Returns (bias_zero, in_maps)."""
    f = lambda k: np.asarray(inputs[k], np.float32)
    x, context = f("x"), f("context")
    wqkv, w_attn_out = f("wqkv"), f("w_attn_out")
    ln1_g, ln1_b = f("ln1_g"), f("ln1_b")
    wq_c, wk_c, wv_c, w_cross_out = (f("wq_c"), f("wk_c"), f("wv_c"),
                                     f("w_cross_out"))
    ln2_g, ln2_b = f("ln2_g"), f("ln2_b")
    w_gate, w_up, w_down = f("w_gate"), f("w_up"), f("w_down")
    ln3_g, ln3_b = f("ln3_g"), f("ln3_b")
    height, width, frames = (int(inputs["height"]), int(inputs["width"]),
                             int(inputs["frames"]))

    bias_zero = bool((ln3_b == 0).all())
    sc = DH ** -0.25

    def fold(W, g, b, scale=1.0):
        Wg = g[:, None] * W * scale
        c0 = -Wg.sum(axis=0)
        c1 = b @ W * scale
        return Wg, np.stack([c0, c1]).astype(BF16)

    wqkv_g, cqkv = fold(wqkv, ln1_g, ln1_b)
    wqkv_g[:, :C] *= sc
    wqkv_g[:, C:2 * C] *= sc
    cqkv[:, :2 * C] *= BF16(sc)
    wqc_g, cqc = fold(wq_c, ln2_g, ln2_b, sc)
    wkc_s = (wk_c * sc).astype(BF16)
    # LN3: mean handled on-chip; fold only the gain.
    wg_g = (ln3_g[:, None] * w_gate).astype(BF16)
    wu_g = (ln3_g[:, None] * w_up).astype(BF16)

    cos, sin = _rope_tables(DH, height, width, frames)
    sinp = sin.copy()
    sinp[:, :DH // 2] *= -1.0
    P = _qk_perm()
    valid = P >= 0
    Pc = np.where(valid, P, 0)
    cosP = np.where(valid[None, :], cos[:, Pc], 0.0)
    sinP = np.where(valid[None, :], sinp[:, Pc], 0.0)
    cosT = np.ascontiguousarray(cosP.T / 64.0).astype(BF16)
    # sin is read at raw's partition base (SB inputs must share it), so
    # pre-swap columns: sin_sb[d] = sinP[d^64], giving
    # out[d] = raw[d]*cosP[d] + raw[d^64]*sin_sb[d^64] = ... + raw[d^64]*sinP[d]
    sinT = np.ascontiguousarray(
        sinP[:, np.arange(DP) ^ 64].T / 64.0).astype(BF16)

    def permute_qk(Wh):  # [rows, DH] -> [rows, DP] permuted+padded
        out = np.zeros((Wh.shape[0], DP), Wh.dtype)
        out[:, valid] = Wh[:, Pc[valid]]
        return out

    tri = np.triu(np.ones((128, 128), np.float32)).astype(BF16)
    ones128 = np.ones((128, 128), np.float32).astype(BF16)

    xT = np.ascontiguousarray(x.transpose(0, 2, 1))          # [B, C, S]
    ctxT = np.ascontiguousarray(context.transpose(0, 2, 1))  # [B, C, CTX]

    # pre-tiled streamed weights (shared across cores)
    wqc_tl = np.ascontiguousarray(
        wqc_g.astype(BF16).reshape(NCT, 128, H, DH).transpose(
            1, 2, 0, 3).reshape(128, H * NCT * DH))
    wo_tl = _tile6(w_attn_out.astype(BF16), NCT)
    wco_tl = _tile6(w_cross_out.astype(BF16), NCT)
    FP8 = ml_dtypes.float8_e4m3fn

    def pack_dr(W, nb, mw=128):
        # [K, nb*mw] -> [128, nb*(K/256)*2*mw] fp8 DoubleRow blocks.
        # Slot (p, parity r) holds contraction row kp*256 + r*128 + p,
        # matching how the kernel packs activation pairs on-chip.
        K = W.shape[0]
        KP = K // 256
        t = W.reshape(KP, 2, 128, nb, mw)           # [kp, r, p, b, m]
        t = t.transpose(2, 3, 0, 1, 4)              # [p, b, kp, r, m]
        return np.ascontiguousarray(
            t.reshape(128, nb * KP * 2 * mw)).astype(FP8)

    wg_tl = pack_dr(np.float32(64.0) * wg_g.astype(np.float32), NIT)
    wu_tl = pack_dr(np.float32(64.0) * wu_g.astype(np.float32), NIT)
    wd_tl = pack_dr(np.float32(64.0) * w_down, NCT)

    def xtile(xb):  # [C, S] -> [128, NMG*NCT*MG], block (g, c)
        return np.ascontiguousarray(
            xb.reshape(NCT, 128, NMG, MG).transpose(1, 2, 0, 3).reshape(
                128, NMG * NCT * MG))

    def rowtile(W, w):  # [C, w] -> [128, NCT*w], block c at cols c*w
        return np.ascontiguousarray(
            W.reshape(NCT, 128, w).transpose(1, 0, 2).reshape(128, NCT * w))

    shared = dict(
        cosT=cosT, sinT=sinT, tri=tri, ones_in=ones128,
        x_bf0=xtile(xT[0].astype(BF16)), x_bf1=xtile(xT[1].astype(BF16)),
        wo_t=wo_tl, wqc_t=wqc_tl, wco_t=wco_tl,
        wkc=rowtile(wkc_s, C), wvc=rowtile(wv_c.astype(BF16), C),
        wg_t=wg_tl, wu_t=wu_tl, wd_t=wd_tl,
        cqc=cqc,
    )
    if not bias_zero:
        shared["cg"] = (ln3_b @ w_gate).astype(BF16)[None, :]
        shared["cu"] = (ln3_b @ w_up).astype(BF16)[None, :]
    in_maps = []
    for core in range(NCORES):
        h = core                      # head owned in phases A/B
        b, gq = core // 4, core % 4   # batch/token-group in phases D/E
        m = dict(shared)
        m["x_own"] = np.ascontiguousarray(xT[b][:, gq * TG:(gq + 1) * TG])
        m["ctx_bf"] = rowtile(ctxT[b].astype(BF16), CTX)
        qs = slice(DH * h, DH * (h + 1))
        wq_h = permute_qk(wqkv_g[:, :C][:, qs]).astype(np.float32)
        wk_h = permute_qk(wqkv_g[:, C:2 * C][:, qs]).astype(np.float32)
        wv_h = wqkv_g[:, 2 * C:][:, qs].astype(np.float32)
        m["wq_p"] = pack_dr(64.0 * wq_h, 1, DP)
        m["wk_p"] = pack_dr(64.0 * wk_h, 1, DP)
        m["wv_p"] = pack_dr(64.0 * wv_h, 1, DH)
        m["cq"] = (64.0 * permute_qk(cqkv[:, :C][:, qs].astype(np.float32))
                   ).astype(BF16)
        m["ck"] = (64.0 * permute_qk(cqkv[:, C:2 * C][:, qs].astype(
            np.float32))).astype(BF16)
        m["cv"] = np.ascontiguousarray(
            64.0 * cqkv[:, 2 * C:][:, qs].astype(np.float32)).astype(BF16)
        in_maps.append(m)
    return bias_zero, in_maps


def _get_nc(inputs):
    bias_zero, in_maps = _prep_inputs(inputs)
    key = ("nc", bias_zero)
    if key not in _CACHE:
        _CACHE[key] = _build_program(bias_zero)
    return _CACHE[key], in_maps


def kernel(**inputs):
    from concourse import bass_utils
    nc, in_maps = _get_nc(inputs)
    res = bass_utils.run_bass_kernel_spmd(nc, in_maps,
                                          core_ids=list(range(NCORES)))
    out = np.empty((B, C, S), np.float32)
    for core in range(NCORES):
        b, g = core // 4, core % 4
        out[b][:, g * TG:(g + 1) * TG] = res.results[core]["out_x"]
    return np.ascontiguousarray(out.transpose(0, 2, 1))


# revision 77
# speedup vs baseline: 1.0748x; 1.0263x over previous
"""Causal3DTransformerBlock on 8 TRN2 NeuronCores.

Sharding: self-attention is head-parallel with ONE head x BOTH batches per
core (core h owns head h).  A single 8-core AllToAll redistributes the
attention outputs to token-parallel (core j: batch j//4, tokens
(j%4)*512 .. +512); every A2A block is real data and the gathered rows are
head-major on every core, so the out-projection (full C contraction) runs
after the exchange with the unpermuted w_attn_out.  Cross-attention and the
SwiGLU FFN are token-parallel (no further collectives).

Compute dtype bf16 (fp32 PSUM accumulation, fp32 residual stream).
LayerNorm gains fold into weights host-side; mean/bias enter as a K=2
rank-1 correction matmul, except the FFN where the mean is subtracted on
DVE (the bias path compiles only when ln3_b is nonzero).
Softmax: raw exp (1/sqrt(dh) folded into wq/wk), denominator accumulated
by an extra ones-row in V, normalization via DVE fast reciprocal +
ones-matmul broadcast.  The FFN runs in fp8 (e4m3, weights x64) with
DoubleRow matmuls; SiLU on the ACT engine descales the gate.
Large weights (wqc, wo, wco, wg, wu, wd) are host-pre-tiled and streamed
through small double-buffered SBUF tiles.
"""

import sys

sys.path.insert(0, "/opt/trn_rl_repo")

import os

import numpy as np
import ml_dtypes

BF16 = ml_dtypes.bfloat16
SKIP_COLLECTIVE = bool(int(os.environ.get("K_SKIP_COLLECTIVE", "0")))

B, S, C, CTX, II, H, DH = 2, 2048, 768, 128, 3072, 8, 96
NCORES = 8
TG = 512         # tokens per core after the exchange (A2A block width)
MG = 1024        # megagroup width for self-attention phases
NMG = S // MG    # 2
NKT = S // 128   # 16 key tiles
KPG = MG // 128  # 8 key tiles per megagroup
NCT = C // 128   # 6 feature tiles
NIT = II // 128  # 24 FFN intermediate tiles
DP = 128         # stored (permuted+padded) Q/K head dim
EPS = 1e-5
RG8 = [[0, 1, 2, 3, 4, 5, 6, 7]]

_CACHE = {}


def _build_program(bias_zero, debug=False):
    import concourse.bass as bass
    import concourse.tile as tile
    from concourse import bacc, mybir
    from concourse.alu_op_type import AluOpType as alu

    f32 = mybir.dt.float32
    bf16 = mybir.dt.bfloat16
    AF = mybir.ActivationFunctionType

    nc = bacc.Bacc("TRN2", debug=False, num_devices=NCORES)

    def din(name, shape, dt=bf16):
        return nc.dram_tensor(name, shape, dt, kind="ExternalInput").ap()

    # x^T pre-tiled [128, (g,c)-blocks of MG cols], both batches
    x_bf = [din(f"x_bf{b}", [128, NMG * NCT * MG]) for b in range(B)]
    x_own = din("x_own", [C, TG], f32)    # own 512-token slice, fp32
    ctx_bf = din("ctx_bf", [128, NCT * CTX])  # own batch ctx^T, pre-tiled
    cosT = din("cosT", [DP, S])
    sinT = din("sinT", [DP, S])           # sign-folded, partner-swapped (^64)
    tri = din("tri", [128, 128])          # triu {0,1} mask: [k,q] valid q>=k
    ones_in = din("ones_in", [128, 128])
    # merged [wq|wk|wv] pre-tiled: block c at cols c*(DP+DP+DH)
    wqkv = din("wqkv_t", [128, NCT * (2 * DP + DH)])
    wkc = din("wkc", [128, NCT * C])      # pre-tiled, block c at cols c*C
    wvc = din("wvc", [128, NCT * C])
    # pre-tiled [128, blocks]: see _prep_inputs for the layouts
    wo_t = din("wo_t", [128, NCT * NCT * 128])
    wqc_t = din("wqc_t", [128, H * NCT * DH])
    wco_t = din("wco_t", [128, NCT * NCT * 128])
    # FFN weights in fp8 (x64 scaled), DoubleRow-packed: two contraction
    # rows interleaved along the free dim; block (it, cpair) is 256 cols
    fp8 = mybir.dt.float8e4
    wg_t = din("wg_t", [128, NIT * 3 * 256], fp8)
    wu_t = din("wu_t", [128, NIT * 3 * 256], fp8)
    wd_t = din("wd_t", [128, NCT * 12 * 256], fp8)
    cq = din("cq", [2, DP])               # corrections: row0=-colsum(W'), row1=b@W
    ck = din("ck", [2, DP])
    cv = din("cv", [2, DH])
    cqc = din("cqc", [2, C])
    if not bias_zero:
        cg = din("cg", [1, II])           # b@Wg (bias variant only)
        cu = din("cu", [1, II])

    out_x = nc.dram_tensor("out_x", [C, TG], f32, kind="ExternalOutput").ap()
    if debug:
        dbg_ai = nc.dram_tensor("dbg_ai", [H * DH, TG], bf16,
                                kind="ExternalOutput").ap()
        dbg_ao = nc.dram_tensor("dbg_ao", [H * DH, TG], bf16,
                                kind="ExternalOutput").ap()
        dbg_x2 = nc.dram_tensor("dbg_x2", [C, TG], f32,
                                kind="ExternalOutput").ap()
        dbg_x3 = nc.dram_tensor("dbg_x3", [C, TG], f32,
                                kind="ExternalOutput").ap()

    with tile.TileContext(nc) as tc:
        with (
            tc.tile_pool(name="const", bufs=1) as cpool,
            tc.tile_pool(name="resid", bufs=1) as rpool,
            tc.tile_pool(name="work", bufs=2) as wpool,
            tc.tile_pool(name="stat", bufs=1) as spool,
            tc.tile_pool(name="dram", bufs=1, space="DRAM") as dpool,
        ):
            # ---- const APs for activation bias ----
            czero = cpool.tile([128, 1], f32, tag="czero", name="czero")
            nc.vector.memset(czero[:], 0.0)
            nc.const_aps.aps[(f32, 0.0)] = czero[:]
            ceps = cpool.tile([128, 1], f32, tag="ceps", name="ceps")
            nc.vector.memset(ceps[:], EPS)
            nc.const_aps.aps[(f32, EPS)] = ceps[:]
            ones_sb = cpool.tile([128, 128], bf16, tag="ones", name="ones")
            nc.sync.dma_start(ones_sb[:], ones_in)
            tri_sb = cpool.tile([128, 128], bf16, tag="tri", name="tri")
            nc.sync.dma_start(tri_sb[:], tri)


            def mmF(ps, lhsT, rhs, c0, c1, start, stop):
                """matmul on cols [c0:c1) of ps/rhs, split at the PSUM bank
                boundary (512 f32 cols)."""
                pts = sorted({c0, c1} | ({512} if c0 < 512 < c1 else set()))
                for a, b in zip(pts, pts[1:]):
                    nc.tensor.matmul(ps[:, a:b], lhsT, rhs[:, a:b],
                                     start=start, stop=stop)

            # ---------------- shared LN helper ----------------
            def layernorm_stats(xt, ps_stat, W, want_mu_bc, xsq_dve=0,
                                tags=("s1", "s2")):
                """xt: accessor c -> [128, W] AP.  xsq on ACT Square (same
                table set as Ln/Exp) except the first `xsq_dve` tiles on
                DVE for balance."""
                s1 = ps_stat.tile([128, W], f32, tag=tags[0], name="s1")
                s2 = ps_stat.tile([128, W], f32, tag=tags[1], name="s2")
                for c in range(NCT):
                    xsq = wpool.tile([128, W], bf16, tag=f"xsq{c % 2}",
                                     name="xsq", bufs=1)
                    if c < xsq_dve:
                        nc.vector.tensor_tensor(xsq[:], xt(c), xt(c),
                                                alu.mult)
                    else:
                        nc.scalar.square(xsq[:], xt(c))
                    mmF(s1, ones_sb[:], xt(c), 0, W,
                        c == 0, c == NCT - 1)
                    mmF(s2, ones_sb[:], xsq, 0, W,
                        c == 0, c == NCT - 1)
                t_mu = spool.tile([128, W], f32, tag="t_mu", name="t_mu")
                t_m2 = spool.tile([128, W], f32, tag="t_m2", name="t_m2")
                t_v = spool.tile([128, W], f32, tag="t_mu", name="t_v")
                t_ln = spool.tile([128, W], f32, tag="t_m2", name="t_ln")
                nc.vector.tensor_scalar(t_mu[:], s1[:], 1.0 / C, None,
                                        alu.mult)
                nc.vector.tensor_tensor(t_m2[:], t_mu[:], t_mu[:], alu.mult)
                nc.vector.scalar_tensor_tensor(t_v[:], s2[:], 1.0 / C,
                                               t_m2[:], alu.mult,
                                               alu.subtract)
                nc.scalar.activation(t_ln[:], t_v[:], AF.Ln, bias=EPS)
                b1 = spool.tile([128, W], bf16, tag="b1", name="b1", bufs=2)
                nc.scalar.activation(b1[:], t_ln[:], AF.Exp, scale=-0.5)
                corr = spool.tile([2, W], bf16, tag="corr", name="corr",
                                  bufs=2)
                mu_bf = spool.tile([1, W], bf16, tag="mu_bf", name="mu_bf")
                nc.vector.tensor_scalar(mu_bf[:], s1[0:1, :], 1.0 / C, None,
                                        alu.mult)
                nc.vector.memset(corr[0:2, :], 1.0)
                nc.vector.tensor_tensor(corr[0:1, :], mu_bf[:], b1[0:1, :],
                                        alu.mult)
                mu_bc = None
                if want_mu_bc:
                    mu_bc = spool.tile([128, W], bf16, tag="mu_bc",
                                       name="mu_bc")
                    nc.vector.tensor_scalar(mu_bc[:], s1[:], 1.0 / C, None,
                                            alu.mult)
                return b1, corr, mu_bc

            # persistent small inputs (merged DMAs)
            WQKV = 2 * DP + DH
            wqkv_sb = cpool.tile([128, NCT * WQKV], bf16, tag="wqkv",
                                 name="wqkv")
            nc.sync.dma_start(wqkv_sb[:], wqkv)
            wq_sb = [wqkv_sb[:, c * WQKV:c * WQKV + DP] for c in range(NCT)]
            wk_sb = [wqkv_sb[:, c * WQKV + DP:c * WQKV + 2 * DP]
                     for c in range(NCT)]
            wv_sb = [wqkv_sb[:, c * WQKV + 2 * DP:(c + 1) * WQKV]
                     for c in range(NCT)]
            cq_sb = cpool.tile([2, DP], bf16, tag="cq", name="cq")
            ck_sb = cpool.tile([2, DP], bf16, tag="ck", name="ck")
            cv_sb = cpool.tile([2, DH], bf16, tag="cv", name="cv")
            nc.sync.dma_start(cq_sb[:], cq)
            nc.sync.dma_start(ck_sb[:], ck)
            nc.sync.dma_start(cv_sb[:], cv)
            ctx_sb = cpool.tile([128, NCT * CTX], bf16, tag="ctx", name="ctx")
            nc.sync.dma_start(ctx_sb[:], ctx_bf)
            ctxa = lambda c: ctx_sb[:, c * CTX:(c + 1) * CTX]
            cqc_sb = cpool.tile([2, C], bf16, tag="cqc", name="cqc")
            nc.sync.dma_start(cqc_sb[:], cqc)
            xo_sb = [rpool.tile([128, TG], f32, tag=f"xo{c}", name=f"xo{c}")
                     for c in range(NCT)]
            for c in range(NCT):
                nc.sync.dma_start(xo_sb[c][:], x_own[c * 128:(c + 1) * 128, :])

            x2 = [rpool.tile([128, TG], f32, tag=f"x2_{c}", name=f"x2_{c}")
                  for c in range(NCT)]
            x3 = [rpool.tile([128, TG], f32, tag=f"xo{c}", name=f"x3_{c}")
                  for c in range(NCT)]

            a2a_in = dpool.tile([H * DH, TG], bf16, name="a2a_in")
            a2a_out = dpool.tile([H * DH, TG], bf16, name="a2a_out")

            # ============== Phases A+B (pool scoped, freed after) ==========
            with tc.tile_pool(name="attn", bufs=1) as apool:
                cos_sb = [apool.tile([DP, MG], bf16, tag=f"cos{g}",
                                     name=f"cos{g}") for g in range(NMG)]
                sin_sb = [apool.tile([DP, MG], bf16, tag=f"sin{g}",
                                     name=f"sin{g}") for g in range(NMG)]
                # x is host-pre-tiled [128, NMG*NCT*MG]: one DMA per (b,g)
                xbf = [[apool.tile([128, NCT * MG], bf16, tag="xbf",
                                   name="xbf", bufs=2) for _ in range(NMG)]
                       for _ in range(B)]
                nc.sync.dma_start(xbf[0][0][:], x_bf[0][:, 0:NCT * MG])
                for g in range(NMG):
                    nc.sync.dma_start(cos_sb[g][:],
                                      cosT[:, g * MG:(g + 1) * MG])
                    nc.sync.dma_start(sin_sb[g][:],
                                      sinT[:, g * MG:(g + 1) * MG])
                nc.sync.dma_start(xbf[0][1][:],
                                  x_bf[0][:, NCT * MG:2 * NCT * MG])
                # x(1,0)/x(1,1) are issued inside the phase-A loop so their
                # buffer-reuse waits don't head-of-line-block the Sync queue

                q_sb = [[apool.tile([DP, MG], bf16, tag=f"q{bb}_{g}",
                                    name=f"q{bb}_{g}") for g in range(NMG)]
                        for bb in range(B)]
                k_sb = [[apool.tile([DP, MG], bf16, tag=f"k{bb}_{g}",
                                    name=f"k{bb}_{g}") for g in range(NMG)]
                        for bb in range(B)]
                v_sb = [[apool.tile([128, DH + 1], bf16, tag=f"v{bb}_{t}",
                                    name=f"v{bb}_{t}") for t in range(NKT)]
                        for bb in range(B)]
                o_sb = [[apool.tile([DH, MG], bf16, tag=f"o{bb}_{g}",
                                    name=f"o{bb}_{g}") for g in range(NMG)]
                        for bb in range(B)]

                # ---- Phase A: LN1 + QKV + RoPE ----
                with (
                    tc.tile_pool(name="ps_statA", bufs=1,
                                 space="PSUM") as ps_sA,
                    tc.tile_pool(name="ps_projA", bufs=2,
                                 space="PSUM") as ps_pA,
                ):
                    for bb in range(B):
                        for g in range(NMG):
                            if (bb, g) in ((0, 1), (1, 0)):
                                nb, ng = (1, 0) if g == 1 else (1, 1)
                                nc.sync.dma_start(
                                    xbf[nb][ng][:],
                                    x_bf[nb][:, ng * NCT * MG:
                                             (ng + 1) * NCT * MG])
                            xt = xbf[bb][g]
                            xa = lambda c: xt[:, c * MG:(c + 1) * MG]
                            b1, corr, _ = layernorm_stats(xa, ps_sA, MG,
                                                          False, xsq_dve=2)
                            z = [apool.tile([128, MG], bf16, tag=f"z{c}",
                                            name="z", bufs=2)
                                 for c in range(NCT)]
                            for c in range(NCT):
                                nc.vector.tensor_tensor(z[c][:], xa(c),
                                                        b1[:], alu.mult)
                            for wt, ct, dst in ((wq_sb, cq_sb, q_sb),
                                                (wk_sb, ck_sb, k_sb)):
                                ps = ps_pA.tile([DP, MG], f32, tag="proj",
                                                name="ps")
                                for c in range(NCT):
                                    mmF(ps, wt[c][:], z[c], 0, MG,
                                        c == 0, False)
                                mmF(ps, ct[:], corr, 0, MG, False, True)
                                raw = apool.tile([DP, MG], bf16, tag="qkraw",
                                                 name="raw", bufs=2)
                                nc.scalar.copy(raw[:], ps[:])
                                # RoPE: out[d] = raw[d]*cos + raw[d^64]*sin'
                                t1 = apool.tile([DP, MG], bf16, tag="rope1",
                                                name="t1", bufs=2)
                                t2 = apool.tile([DP, MG], bf16, tag="rope2",
                                                name="t2", bufs=2)
                                eng = nc.gpsimd if dst is q_sb else nc.vector
                                nc.vector.tensor_tensor(t1[:], raw[:],
                                                        cos_sb[g][:],
                                                        alu.mult)
                                eng.tensor_tensor(t2[0:64, :],
                                                  raw[64:128, :],
                                                  sin_sb[g][64:128, :],
                                                  alu.mult)
                                eng.tensor_tensor(t2[64:128, :],
                                                  raw[0:64, :],
                                                  sin_sb[g][0:64, :],
                                                  alu.mult)
                                nc.vector.tensor_tensor(dst[bb][g][:], t1[:],
                                                        t2[:], alu.add)
                            for tt in range(KPG):
                                kt = g * KPG + tt
                                ps = ps_pA.tile([128, DH], f32, tag="proj",
                                                name="ps")
                                for c in range(NCT):
                                    nc.tensor.matmul(
                                        ps[:],
                                        z[c][:, tt * 128:(tt + 1) * 128],
                                        wv_sb[c][:], start=(c == 0),
                                        stop=False)
                                nc.tensor.matmul(
                                    ps[:], corr[:, tt * 128:(tt + 1) * 128],
                                    cv_sb[:], start=False, stop=True)
                                nc.scalar.copy(v_sb[bb][kt][:, 0:DH],
                                               ps[:])
                                nc.vector.memset(v_sb[bb][kt][:, DH:DH + 1],
                                                 1.0)

                # ---- Phase B: causal attention ----
                with tc.tile_pool(name="ps_attn", bufs=1,
                                  space="PSUM") as ps_at:
                    with nc.allow_low_precision(
                            reason="softmax reciprocal bf16"):
                        for bb in range(B):
                            for qg in range(NMG):
                                o_ps = ps_at.tile([DH + 1, MG], f32, tag="av",
                                                  name="o_ps", bufs=1)
                                nkt = KPG * qg + KPG
                                for kt in range(nkt):
                                    p = kt - KPG * qg
                                    q0 = 0 if p < 0 else p * 128
                                    s_ps = ps_at.tile([128, MG], f32,
                                                      tag="scores",
                                                      name="s_ps", bufs=2)
                                    mmF(s_ps,
                                        k_sb[bb][kt // KPG][
                                            :, (kt % KPG) * 128:
                                            (kt % KPG + 1) * 128],
                                        q_sb[bb][qg], q0, MG, True, True)
                                    pt = apool.tile([128, MG], bf16,
                                                    tag="ptB", name="pt",
                                                    bufs=2)
                                    nc.scalar.activation(pt[:, q0:MG],
                                                         s_ps[:, q0:MG],
                                                         AF.Exp)
                                    if p >= 0:
                                        nc.vector.tensor_tensor(
                                            pt[:, q0:q0 + 128],
                                            pt[:, q0:q0 + 128],
                                            tri_sb[:], alu.mult)
                                    mmF(o_ps, v_sb[bb][kt][:], pt, q0, MG,
                                        kt == 0, kt == nkt - 1)
                                den = spool.tile([1, MG], f32, tag="den",
                                                 name="den", bufs=2)
                                nc.vector.tensor_copy(den[:],
                                                      o_ps[DH:DH + 1, :])
                                rcp = spool.tile([1, MG], f32, tag="rcp",
                                                 name="rcp", bufs=2)
                                nc.vector.reciprocal_approx_fast(rcp[:],
                                                                 den[:])
                                rcpb = spool.tile([1, MG], bf16, tag="rcpb",
                                                  name="rcpb", bufs=2)
                                nc.vector.tensor_copy(rcpb[:], rcp[:])
                                b_ps = ps_at.tile([128, MG], f32, tag="bcast",
                                                  name="b_ps", bufs=1)
                                mmF(b_ps, ones_sb[0:1, :], rcpb, 0, MG,
                                    True, True)
                                b_sb = apool.tile([128, MG], bf16, tag="bsb",
                                                  name="b_sb", bufs=2)
                                nc.vector.tensor_copy(b_sb[:], b_ps[:])
                                nc.vector.tensor_tensor(o_sb[bb][qg][:],
                                                        o_ps[0:DH, :],
                                                        b_sb[0:DH, :],
                                                        alu.mult)
                                for half in range(2):
                                    j = bb * 4 + qg * 2 + half
                                    nc.sync.dma_start(
                                        a2a_in[j * DH:(j + 1) * DH, :],
                                        o_sb[bb][qg][:, half * TG:
                                                     (half + 1) * TG])

            # fire the exchange; fill the wait with ctx-side cross-attn work
            if SKIP_COLLECTIVE:
                nc.sync.dma_start(a2a_out[:], a2a_in[:])
            else:
                nc.gpsimd.collective_compute(
                    "AllToAll", alu.bypass, replica_groups=RG8,
                    ins=[a2a_in.opt()], outs=[a2a_out.opt()])
            if debug:
                nc.sync.dma_start(dbg_ai, a2a_in[:])
                nc.sync.dma_start(dbg_ao, a2a_out[:])

            with (
                tc.tile_pool(name="cross", bufs=1) as xpool,
                tc.tile_pool(name="wstr", bufs=1) as wstr,
                tc.tile_pool(name="ps_projD", bufs=2, space="PSUM") as ps_pD,
                tc.tile_pool(name="ps_attn2", bufs=1, space="PSUM") as ps_at2,
            ):
                # ---- context-side projections (independent of the A2A) ----
                wkc_sb = xpool.tile([128, NCT * C], bf16, tag="wkc",
                                    name="wkc")
                wvc_sb = xpool.tile([128, NCT * C], bf16, tag="wvc",
                                    name="wvc")
                nc.sync.dma_start(wkc_sb[:], wkc)
                nc.sync.dma_start(wvc_sb[:], wvc)
                kc_sb = [xpool.tile([DH, CTX], bf16, tag=f"kc{h}",
                                    name=f"kc{h}") for h in range(H)]
                for h in range(H):
                    ps = ps_pD.tile([DH, CTX], f32, tag="proj", name="ps")
                    for c in range(NCT):
                        nc.tensor.matmul(
                            ps[:],
                            wkc_sb[:, c * C + h * DH:c * C + (h + 1) * DH],
                            ctxa(c), start=(c == 0), stop=(c == NCT - 1))
                    nc.vector.tensor_copy(kc_sb[h][:], ps[:])
                vc_sb = xpool.tile([128, H * (DH + 1)], bf16, tag="vc",
                                   name="vc")
                for half in range(2):
                    ps = ps_pD.tile([128, C // 2], f32, tag="proj", name="ps")
                    for c in range(NCT):
                        nc.tensor.matmul(
                            ps[:], ctxa(c),
                            wvc_sb[:, c * C + half * 384:
                                   c * C + (half + 1) * 384],
                            start=(c == 0), stop=(c == NCT - 1))
                    dv = vc_sb[:].rearrange("p (h d) -> p h d", h=H)[
                        :, half * 4:(half + 1) * 4, 0:DH]
                    sv = ps[:].rearrange("p (h d) -> p h d", h=4)
                    nc.vector.tensor_copy(dv, sv)
                nc.vector.memset(
                    vc_sb[:].rearrange("p (h d) -> p h d",
                                       h=H)[:, :, DH:DH + 1], 1.0)

                # ---- after the A2A: gather heads, out-project, residual ----
                oa = [xpool.tile([128, TG], bf16, tag=f"oa{k}",
                                 name=f"oa{k}") for k in range(NCT)]
                for k in range(NCT):
                    nc.sync.dma_start(oa[k][:],
                                      a2a_out[k * 128:(k + 1) * 128, :])
                x2bf = [xpool.tile([128, TG], bf16, tag=f"x2bf{c}",
                                   name=f"x2bf{c}") for c in range(NCT)]
                for ot in range(NCT):
                    wot = wstr.tile([128, NCT * 128], bf16, tag="wot",
                                    name="wot", bufs=3)
                    nc.sync.dma_start(wot[:],
                                      wo_t[:, ot * C:(ot + 1) * C])
                    ps = ps_pD.tile([128, TG], f32, tag="proj", name="ps")
                    for k in range(NCT):
                        nc.tensor.matmul(ps[:],
                                         wot[:, k * 128:(k + 1) * 128],
                                         oa[k][:], start=(k == 0),
                                         stop=(k == NCT - 1))
                    nc.vector.tensor_tensor(x2[ot][:], ps[:], xo_sb[ot][:],
                                            alu.add)
                    nc.scalar.copy(x2bf[ot][:], x2[ot][:])
                    if debug:
                        nc.sync.dma_start(dbg_x2[ot * 128:(ot + 1) * 128, :],
                                          x2[ot][:])

                # ---- LN2 + cross-attention ----
                b1, corr, _ = layernorm_stats(lambda c: x2bf[c][:], ps_pD,
                                              TG, False, tags=("proj", "proj"))
                z2 = [xpool.tile([128, TG], bf16, tag=f"z2_{c}",
                                 name=f"z2_{c}") for c in range(NCT)]
                for c in range(NCT):
                    nc.vector.tensor_tensor(z2[c][:], x2bf[c][:], b1[:],
                                            alu.mult)
                ocfm = [xpool.tile([128, TG], bf16, tag=f"oa{c}",
                                   name=f"ocfm{c}") for c in range(NCT)]
                wqct = [wstr.tile([128, NCT * DH], bf16, tag="wqct",
                                  name="wqct", bufs=4) for h in range(H)]
                for h in range(H):
                    nc.sync.dma_start(
                        wqct[h][:],
                        wqc_t[:, h * NCT * DH:(h + 1) * NCT * DH])
                with nc.allow_low_precision(reason="softmax reciprocal bf16"):
                    for h in range(H):
                        qc_ps = ps_pD.tile([DH, TG], f32, tag="proj",
                                           name="ps")
                        for c in range(NCT):
                            nc.tensor.matmul(
                                qc_ps[:], wqct[h][:, c * DH:(c + 1) * DH],
                                z2[c][:], start=(c == 0), stop=False)
                        nc.tensor.matmul(qc_ps[:],
                                         cqc_sb[:, h * DH:(h + 1) * DH],
                                         corr[:], start=False, stop=True)
                        qc = wpool.tile([DH, TG], bf16, tag="qc", name="qc")
                        nc.vector.tensor_copy(qc[:], qc_ps[:])
                        s_ps = ps_at2.tile([CTX, TG], f32, tag="scores",
                                           name="s_ps", bufs=2)
                        nc.tensor.matmul(s_ps[:], kc_sb[h][:], qc[:],
                                         start=True, stop=True)
                        pt = wpool.tile([CTX, TG], bf16, tag="ptD",
                                        name="pt")
                        nc.scalar.activation(pt[:], s_ps[:], AF.Exp)
                        o_ps = ps_at2.tile([DH + 1, TG], f32, tag="av",
                                           name="o_ps", bufs=2)
                        nc.tensor.matmul(
                            o_ps[:],
                            vc_sb[:, h * (DH + 1):(h + 1) * (DH + 1)],
                            pt[:], start=True, stop=True)
                        den = spool.tile([1, TG], f32, tag="den", name="den",
                                         bufs=2)
                        nc.scalar.copy(den[:], o_ps[DH:DH + 1, :])
                        rcp = spool.tile([1, TG], f32, tag="rcp", name="rcp",
                                         bufs=2)
                        nc.vector.reciprocal_approx_fast(rcp[:], den[:])
                        rcpb = spool.tile([1, TG], bf16, tag="rcpb",
                                          name="rcpb", bufs=2)
                        nc.vector.tensor_copy(rcpb[:], rcp[:])
                        b_ps = ps_at2.tile([128, TG], f32, tag="bcast",
                                           name="b_ps", bufs=1)
                        nc.tensor.matmul(b_ps[:], ones_sb[0:1, :], rcpb[:],
                                         start=True, stop=True)
                        b_sb = wpool.tile([128, TG], bf16, tag="bsbD",
                                          name="b_sb")
                        nc.scalar.copy(b_sb[:], b_ps[:])

                        def _maxn(v):
                            if v % 128 == 0:
                                return 128
                            if v % 64 == 0:
                                return 64
                            return 32
                        pos = 0
                        while pos < DH:
                            r = h * DH + pos
                            c0, off = r // 128, r % 128
                            n = min(_maxn(off), _maxn(pos), DH - pos,
                                    128 - off)
                            nc.vector.tensor_tensor(
                                ocfm[c0][off:off + n, :],
                                o_ps[pos:pos + n, :],
                                b_sb[pos:pos + n, :], alu.mult)
                            pos += n

                x3bf = [xpool.tile([128, TG], bf16, tag=f"z2_{c}",
                                   name=f"x3bf{c}") for c in range(NCT)]
                for ot in range(NCT):
                    wcot = wstr.tile([128, NCT * 128], bf16, tag="wcot",
                                     name="wcot", bufs=3)
                    nc.sync.dma_start(wcot[:],
                                      wco_t[:, ot * C:(ot + 1) * C])
                    ps = ps_pD.tile([128, TG], f32, tag="proj", name="ps")
                    for c in range(NCT):
                        nc.tensor.matmul(ps[:],
                                         wcot[:, c * 128:(c + 1) * 128],
                                         ocfm[c][:], start=(c == 0),
                                         stop=(c == NCT - 1))
                    nc.vector.tensor_tensor(x3[ot][:], ps[:], x2[ot][:],
                                            alu.add)
                    nc.scalar.copy(x3bf[ot][:], x3[ot][:])
                    if debug:
                        nc.sync.dma_start(dbg_x3[ot * 128:(ot + 1) * 128, :],
                                          x3[ot][:])

                # ---- LN3 (z3 mean-subtracted on DVE, fp8 DoubleRow pack) --
                b1, corr, mu_bc = layernorm_stats(lambda c: x3bf[c][:],
                                                  ps_pD, TG, True,
                                                  tags=("proj", "proj"))
                z3p = [xpool.tile([128, 2 * TG], fp8, tag=f"z3p{cp}",
                                  name=f"z3p{cp}") for cp in range(3)]
                for c in range(NCT):
                    zt = wpool.tile([128, TG], bf16, tag="zt", name="zt")
                    nc.vector.tensor_tensor(zt[:], x3bf[c][:], mu_bc[:],
                                            alu.subtract)
                    dst = z3p[c // 2][:, (c % 2) * TG:(c % 2 + 1) * TG]
                    nc.vector.tensor_tensor(dst, zt[:], b1[:], alu.mult)
                if not bias_zero:
                    onerow = xpool.tile([1, TG], bf16, tag="onerow",
                                        name="onerow")
                    nc.vector.memset(onerow[:], 1.0)
                    cg_sb = xpool.tile([1, II], bf16, tag="cg", name="cg")
                    cu_sb = xpool.tile([1, II], bf16, tag="cu", name="cu")
                    nc.sync.dma_start(cg_sb[:], cg)
                    nc.sync.dma_start(cu_sb[:], cu)

                # ---- Phase E: SwiGLU FFN, fp8 DoubleRow (weights x64) ----
                hh = [xpool.tile([128, 2 * TG], fp8, tag=f"hh{ip}",
                                 name=f"hh{ip}") for ip in range(12)]
                DR = mybir.MatmulPerfMode.DoubleRow
                if True:
                    for it in range(NIT):
                        wgt = wstr.tile([128, 3 * 256], fp8, tag="wgt",
                                        name="wgt", bufs=3)
                        wut = wstr.tile([128, 3 * 256], fp8, tag="wut",
                                        name="wut", bufs=3)
                        nc.sync.dma_start(
                            wgt[:], wg_t[:, it * 768:(it + 1) * 768])
                        nc.sync.dma_start(
                            wut[:], wu_t[:, it * 768:(it + 1) * 768])
                        g_ps = ps_at2.tile([128, TG], f32, tag="scores",
                                           name="g_ps", bufs=2)
                        u_ps = ps_pD.tile([128, TG], f32, tag="proj",
                                          name="u_ps")
                        for cp in range(3):
                            last = (cp == 2) and bias_zero
                            zr = z3p[cp][:].rearrange("p (r t) -> p r t",
                                                      r=2)
                            for w_, ps_ in ((wgt, g_ps), (wut, u_ps)):
                                wr = w_[:, cp * 256:(cp + 1) * 256].rearrange(
                                    "p (r m) -> p r m", r=2)
                                nc.tensor.matmul(ps_[:], wr, zr,
                                                 start=(cp == 0), stop=last,
                                                 perf_mode=DR)
                        if not bias_zero:
                            nc.tensor.matmul(
                                g_ps[:], cg_sb[:, it * 128:(it + 1) * 128],
                                onerow[:], start=False, stop=True)
                            nc.tensor.matmul(
                                u_ps[:], cu_sb[:, it * 128:(it + 1) * 128],
                                onerow[:], start=False, stop=True)
                        # silu(g_true)*u_true: ACT Silu descales g (x1/64),
                        # the u descale (1/64) folds into the hh write
                        sg = wpool.tile([128, TG], bf16, tag="sg", name="sg")
                        nc.scalar.activation(sg[:], g_ps[:], AF.Silu,
                                             scale=1.0 / 64)
                        hdst = hh[it // 2][:, (it % 2) * TG:
                                           (it % 2 + 1) * TG]
                        nc.vector.scalar_tensor_tensor(hdst, sg[:],
                                                       1.0 / 64, u_ps[:],
                                                       alu.mult, alu.mult)
                if True:
                    for ot in range(NCT):
                        wdt = wstr.tile([128, 12 * 256], fp8, tag="wdt",
                                        name="wdt", bufs=2)
                        nc.sync.dma_start(
                            wdt[:], wd_t[:, ot * 12 * 256:
                                         (ot + 1) * 12 * 256])
                        d_ps = ps_at2.tile([128, TG], f32, tag="scores",
                                           name="d_ps", bufs=2)
                        for ip in range(12):
                            wr = wdt[:, ip * 256:(ip + 1) * 256].rearrange(
                                "p (r m) -> p r m", r=2)
                            hr = hh[ip][:].rearrange("p (r t) -> p r t", r=2)
                            nc.tensor.matmul(d_ps[:], wr, hr,
                                             start=(ip == 0), stop=(ip == 11),
                                             perf_mode=DR)
                        xf = wpool.tile([128, TG], f32, tag="xf", name="xf")
                        nc.vector.scalar_tensor_tensor(xf[:], d_ps[:],
                                                       1.0 / 64, x3[ot][:],
                                                       alu.mult, alu.add)
                        nc.sync.dma_start(out_x[ot * 128:(ot + 1) * 128, :],
                                          xf[:])

    nc.compile()
    return nc


def _rope_tables(head_dim, height, width, frames, base=10000.0):
    d = head_dim // 3
    dx, dy, dt_ = d, d, head_dim - 2 * d

    def freqs(n, dd):
        inv = 1.0 / base ** (np.arange(0, dd, 2, dtype=np.float32) / dd)
        f = np.outer(np.arange(n, dtype=np.float32), inv)
        return np.concatenate([f, f], axis=-1)

    fx, fy, ft = freqs(width, dx), freqs(height, dy), freqs(frames, dt_)
    shp = (frames, height, width)
    cx = np.broadcast_to(np.cos(fx)[None, None, :, :], shp + (dx,))
    sx = np.broadcast_to(np.sin(fx)[None, None, :, :], shp + (dx,))
    cy = np.broadcast_to(np.cos(fy)[None, :, None, :], shp + (dy,))
    sy = np.broadcast_to(np.sin(fy)[None, :, None, :], shp + (dy,))
    ct = np.broadcast_to(np.cos(ft)[:, None, None, :], shp + (dt_,))
    st = np.broadcast_to(np.sin(ft)[:, None, None, :], shp + (dt_,))
    cos = np.concatenate([cx, cy, ct], axis=-1).reshape(-1, head_dim)
    sin = np.concatenate([sx, sy, st], axis=-1).reshape(-1, head_dim)
    return cos.astype(np.float32), sin.astype(np.float32)


def _qk_perm():
    """Stored-index -> original head-dim map (-1 = zero pad), length 128.
    Layout [x1(48) pad16 | x2(48) pad16] puts every rotate-half partner at
    stored index s^64."""
    P = np.full(DP, -1, np.int64)
    P[0:48] = np.arange(0, 48)
    P[64:112] = np.arange(48, 96)
    return P


def _tile6(W, nb):
    """[C, nb*128] -> [128, nb*NCT*128] with block (b, c) at cols
    (b*NCT+c)*128."""
    return np.ascontiguousarray(
        W.reshape(NCT, 128, nb, 128).transpose(1, 2, 0, 3).reshape(
            128, nb * NCT * 128))


def _prep_inputs(inputs):
    """Host-side prep.  Returns (bias_zero, in_maps)."""
    f = lambda k: np.asarray(inputs[k], np.float32)
    x, context = f("x"), f("context")
    wqkv, w_attn_out = f("wqkv"), f("w_attn_out")
    ln1_g, ln1_b = f("ln1_g"), f("ln1_b")
    wq_c, wk_c, wv_c, w_cross_out = (f("wq_c"), f("wk_c"), f("wv_c"),
                                     f("w_cross_out"))
    ln2_g, ln2_b = f("ln2_g"), f("ln2_b")
    w_gate, w_up, w_down = f("w_gate"), f("w_up"), f("w_down")
    ln3_g, ln3_b = f("ln3_g"), f("ln3_b")
    height, width, frames = (int(inputs["height"]), int(inputs["width"]),
                             int(inputs["frames"]))

    bias_zero = bool((ln3_b == 0).all())
    sc = DH ** -0.25

    def fold(W, g, b, scale=1.0):
        Wg = g[:, None] * W * scale
        c0 = -Wg.sum(axis=0)
        c1 = b @ W * scale
        return Wg, np.stack([c0, c1]).astype(BF16)

    wqkv_g, cqkv = fold(wqkv, ln1_g, ln1_b)
    wqkv_g[:, :C] *= sc
    wqkv_g[:, C:2 * C] *= sc
    cqkv[:, :2 * C] *= BF16(sc)
    wqc_g, cqc = fold(wq_c, ln2_g, ln2_b, sc)
    wkc_s = (wk_c * sc).astype(BF16)
    # LN3: mean handled on-chip; fold only the gain.
    wg_g = (ln3_g[:, None] * w_gate).astype(BF16)
    wu_g = (ln3_g[:, None] * w_up).astype(BF16)

    cos, sin = _rope_tables(DH, height, width, frames)
    sinp = sin.copy()
    sinp[:, :DH // 2] *= -1.0
    P = _qk_perm()
    valid = P >= 0
    Pc = np.where(valid, P, 0)
    cosP = np.where(valid[None, :], cos[:, Pc], 0.0)
    sinP = np.where(valid[None, :], sinp[:, Pc], 0.0)
    cosT = np.ascontiguousarray(cosP.T).astype(BF16)
    # sin is read at raw's partition base (SB inputs must share it), so
    # pre-swap columns: sin_sb[d] = sinP[d^64], giving
    # out[d] = raw[d]*cosP[d] + raw[d^64]*sin_sb[d^64] = ... + raw[d^64]*sinP[d]
    sinT = np.ascontiguousarray(sinP[:, np.arange(DP) ^ 64].T).astype(BF16)

    def permute_qk(Wh):  # [rows, DH] -> [rows, DP] permuted+padded
        out = np.zeros((Wh.shape[0], DP), Wh.dtype)
        out[:, valid] = Wh[:, Pc[valid]]
        return out

    tri = np.triu(np.ones((128, 128), np.float32)).astype(BF16)
    ones128 = np.ones((128, 128), np.float32).astype(BF16)

    xT = np.ascontiguousarray(x.transpose(0, 2, 1))          # [B, C, S]
    ctxT = np.ascontiguousarray(context.transpose(0, 2, 1))  # [B, C, CTX]

    # pre-tiled streamed weights (shared across cores)
    wqc_tl = np.ascontiguousarray(
        wqc_g.astype(BF16).reshape(NCT, 128, H, DH).transpose(
            1, 2, 0, 3).reshape(128, H * NCT * DH))
    wo_tl = _tile6(w_attn_out.astype(BF16), NCT)
    wco_tl = _tile6(w_cross_out.astype(BF16), NCT)
    FP8 = ml_dtypes.float8_e4m3fn

    def pack_dr(W, nb):
        # [K, nb*128] -> [128, nb*(K/256)*256] fp8 DoubleRow blocks.
        # Slot (p, parity r) holds contraction row kp*256 + r*128 + p,
        # matching how the kernel packs z3/hh pairs on-chip:
        # lhsT[p, ((b*KP + kp)*128 + m)*2 + r] = W[kp*256 + r*128 + p, b*128+m]
        K = W.shape[0]
        KP = K // 256
        t = W.reshape(KP, 2, 128, nb, 128)          # [kp, r, p, b, m]
        t = t.transpose(2, 3, 0, 1, 4)              # [p, b, kp, r, m]
        return np.ascontiguousarray(t.reshape(128, nb * KP * 256)).astype(FP8)

    wg_tl = pack_dr(np.float32(64.0) * wg_g.astype(np.float32), NIT)
    wu_tl = pack_dr(np.float32(64.0) * wu_g.astype(np.float32), NIT)
    wd_tl = pack_dr(np.float32(64.0) * w_down, NCT)

    def xtile(xb):  # [C, S] -> [128, NMG*NCT*MG], block (g, c)
        return np.ascontiguousarray(
            xb.reshape(NCT, 128, NMG, MG).transpose(1, 2, 0, 3).reshape(
                128, NMG * NCT * MG))

    def rowtile(W, w):  # [C, w] -> [128, NCT*w], block c at cols c*w
        return np.ascontiguousarray(
            W.reshape(NCT, 128, w).transpose(1, 0, 2).reshape(128, NCT * w))

    shared = dict(
        cosT=cosT, sinT=sinT, tri=tri, ones_in=ones128,
        x_bf0=xtile(xT[0].astype(BF16)), x_bf1=xtile(xT[1].astype(BF16)),
        wo_t=wo_tl, wqc_t=wqc_tl, wco_t=wco_tl,
        wkc=rowtile(wkc_s, C), wvc=rowtile(wv_c.astype(BF16), C),
        wg_t=wg_tl, wu_t=wu_tl, wd_t=wd_tl,
        cqc=cqc,
    )
    if not bias_zero:
        shared["cg"] = (ln3_b @ w_gate).astype(BF16)[None, :]
        shared["cu"] = (ln3_b @ w_up).astype(BF16)[None, :]
    in_maps = []
    for core in range(NCORES):
        h = core                      # head owned in phases A/B
        b, gq = core // 4, core % 4   # batch/token-group in phases D/E
        m = dict(shared)
        m["x_own"] = np.ascontiguousarray(xT[b][:, gq * TG:(gq + 1) * TG])
        m["ctx_bf"] = rowtile(ctxT[b].astype(BF16), CTX)
        qs = slice(DH * h, DH * (h + 1))
        wq_h = permute_qk(wqkv_g[:, :C][:, qs]).astype(BF16)
        wk_h = permute_qk(wqkv_g[:, C:2 * C][:, qs]).astype(BF16)
        wv_h = wqkv_g[:, 2 * C:][:, qs].astype(BF16)
        # merged [wq|wk|wv] per 128-row tile: [128, NCT*(2*DP+DH)]
        wqkv_h = np.concatenate([wq_h, wk_h, wv_h], axis=1)  # [C, 352]
        m["wqkv_t"] = np.ascontiguousarray(
            wqkv_h.reshape(NCT, 128, 2 * DP + DH).transpose(1, 0, 2).reshape(
                128, NCT * (2 * DP + DH)))
        m["cq"] = permute_qk(cqkv[:, :C][:, qs].astype(np.float32)
                             ).astype(BF16)
        m["ck"] = permute_qk(cqkv[:, C:2 * C][:, qs].astype(np.float32)
                             ).astype(BF16)
        m["cv"] = np.ascontiguousarray(cqkv[:, 2 * C:][:, qs])
        in_maps.append(m)
    return bias_zero, in_maps


def _get_nc(inputs):
    bias_zero, in_maps = _prep_inputs(inputs)
    key = ("nc", bias_zero)
    if key not in _CACHE:
        _CACHE[key] = _build_program(bias_zero)
    return _CACHE[key], in_maps


def kernel(**inputs):
    from concourse import bass_utils
    nc, in_maps = _get_nc(inputs)
    res = bass_utils.run_bass_kernel_spmd(nc, in_maps,
                                          core_ids=list(range(NCORES)))
    out = np.empty((B, C, S), np.float32)
    for core in range(NCORES):
        b, g = core // 4, core % 4
        out[b][:, g * TG:(g + 1) * TG] = res.results[core]["out_x"]
    return np.ascontiguousarray(out.transpose(0, 2, 1))


# revision 80
# speedup vs baseline: 1.1093x; 1.0320x over previous
"""Causal3DTransformerBlock on 8 TRN2 NeuronCores.

Sharding: self-attention is head-parallel with ONE head x BOTH batches per
core (core h owns head h).  A single 8-core AllToAll redistributes the
attention outputs to token-parallel (core j: batch j//4, tokens
(j%4)*512 .. +512); every A2A block is real data and the gathered rows are
head-major on every core, so the out-projection (full C contraction) runs
after the exchange with the unpermuted w_attn_out.  Cross-attention and the
SwiGLU FFN are token-parallel (no further collectives).

Compute dtype bf16 (fp32 PSUM accumulation, fp32 residual stream).
LayerNorm gains fold into weights host-side; mean/bias enter as a K=2
rank-1 correction matmul, except the FFN where the mean is subtracted on
DVE (the bias path compiles only when ln3_b is nonzero).
Softmax: raw exp (1/sqrt(dh) folded into wq/wk), denominator accumulated
by an extra ones-row in V, normalization via DVE fast reciprocal +
ones-matmul broadcast.  The FFN runs in fp8 (e4m3, weights x64) with
DoubleRow matmuls; SiLU on the ACT engine descales the gate.
Large weights (wqc, wo, wco, wg, wu, wd) are host-pre-tiled and streamed
through small double-buffered SBUF tiles.
"""

import sys

sys.path.insert(0, "/opt/trn_rl_repo")

import os

import numpy as np
import ml_dtypes

BF16 = ml_dtypes.bfloat16
SKIP_COLLECTIVE = bool(int(os.environ.get("K_SKIP_COLLECTIVE", "0")))

B, S, C, CTX, II, H, DH = 2, 2048, 768, 128, 3072, 8, 96
NCORES = 8
TG = 512         # tokens per core after the exchange (A2A block width)
MG = 1024        # megagroup width for self-attention phases
NMG = S // MG    # 2
NKT = S // 128   # 16 key tiles
KPG = MG // 128  # 8 key tiles per megagroup
NCT = C // 128   # 6 feature tiles
NIT = II // 128  # 24 FFN intermediate tiles
DP = 128         # stored (permuted+padded) Q/K head dim
EPS = 1e-5
RG8 = [[0, 1, 2, 3, 4, 5, 6, 7]]

_CACHE = {}


def _build_program(bias_zero, debug=False):
    import concourse.bass as bass
    import concourse.tile as tile
    from concourse import bacc, mybir
    from concourse.alu_op_type import AluOpType as alu

    f32 = mybir.dt.float32
    bf16 = mybir.dt.bfloat16
    AF = mybir.ActivationFunctionType

    nc = bacc.Bacc("TRN2", debug=False, num_devices=NCORES)

    def din(name, shape, dt=bf16):
        return nc.dram_tensor(name, shape, dt, kind="ExternalInput").ap()

    # x^T pre-tiled [128, (g,c)-blocks of MG cols], both batches
    x_bf = [din(f"x_bf{b}", [128, NMG * NCT * MG]) for b in range(B)]
    x_own = din("x_own", [C, TG], f32)    # own 512-token slice, fp32
    ctx_bf = din("ctx_bf", [128, NCT * CTX])  # own batch ctx^T, pre-tiled
    cosT = din("cosT", [DP, S])
    sinT = din("sinT", [DP, S])           # sign-folded, partner-swapped (^64)
    tri = din("tri", [128, 128])          # triu {0,1} mask: [k,q] valid q>=k
    ones_in = din("ones_in", [128, 128])
    # merged [wq|wk|wv] pre-tiled: block c at cols c*(DP+DP+DH)
    wqkv = din("wqkv_t", [128, NCT * (2 * DP + DH)])
    wkc = din("wkc", [128, NCT * C])      # pre-tiled, block c at cols c*C
    wvc = din("wvc", [128, NCT * C])
    # pre-tiled [128, blocks]: see _prep_inputs for the layouts
    wo_t = din("wo_t", [128, NCT * NCT * 128])
    wqc_t = din("wqc_t", [128, H * NCT * DH])
    wco_t = din("wco_t", [128, NCT * NCT * 128])
    # FFN weights in fp8 (x64 scaled), DoubleRow-packed: two contraction
    # rows interleaved along the free dim; block (it, cpair) is 256 cols
    fp8 = mybir.dt.float8e4
    wg_t = din("wg_t", [128, NIT * 3 * 256], fp8)
    wu_t = din("wu_t", [128, NIT * 3 * 256], fp8)
    wd_t = din("wd_t", [128, NCT * 12 * 256], fp8)
    cq = din("cq", [2, DP])               # corrections: row0=-colsum(W'), row1=b@W
    ck = din("ck", [2, DP])
    cv = din("cv", [2, DH])
    cqc = din("cqc", [2, C])
    if not bias_zero:
        cg = din("cg", [1, II])           # b@Wg (bias variant only)
        cu = din("cu", [1, II])

    out_x = nc.dram_tensor("out_x", [C, TG], f32, kind="ExternalOutput").ap()
    if debug:
        dbg_ai = nc.dram_tensor("dbg_ai", [H * DH, TG], bf16,
                                kind="ExternalOutput").ap()
        dbg_ao = nc.dram_tensor("dbg_ao", [H * DH, TG], bf16,
                                kind="ExternalOutput").ap()
        dbg_x2 = nc.dram_tensor("dbg_x2", [C, TG], f32,
                                kind="ExternalOutput").ap()
        dbg_x3 = nc.dram_tensor("dbg_x3", [C, TG], f32,
                                kind="ExternalOutput").ap()

    with tile.TileContext(nc) as tc:
        with (
            tc.tile_pool(name="const", bufs=1) as cpool,
            tc.tile_pool(name="resid", bufs=1) as rpool,
            tc.tile_pool(name="work", bufs=2) as wpool,
            tc.tile_pool(name="stat", bufs=1) as spool,
            tc.tile_pool(name="dram", bufs=1, space="DRAM") as dpool,
        ):
            # ---- const APs for activation bias ----
            czero = cpool.tile([128, 1], f32, tag="czero", name="czero")
            nc.vector.memset(czero[:], 0.0)
            nc.const_aps.aps[(f32, 0.0)] = czero[:]
            ceps = cpool.tile([128, 1], f32, tag="ceps", name="ceps")
            nc.vector.memset(ceps[:], EPS)
            nc.const_aps.aps[(f32, EPS)] = ceps[:]
            xbf00 = None  # placeholder; x(0,0) DMA issued below, first
            ones_sb = cpool.tile([128, 128], bf16, tag="ones", name="ones")
            tri_sb = cpool.tile([128, 128], bf16, tag="tri", name="tri")


            def mmF(ps, lhsT, rhs, c0, c1, start, stop):
                """matmul on cols [c0:c1) of ps/rhs, split at the PSUM bank
                boundary (512 f32 cols)."""
                pts = sorted({c0, c1} | ({512} if c0 < 512 < c1 else set()))
                for a, b in zip(pts, pts[1:]):
                    nc.tensor.matmul(ps[:, a:b], lhsT, rhs[:, a:b],
                                     start=start, stop=stop)

            # ---------------- shared LN helper ----------------
            def layernorm_stats(xt, ps_stat, W, want_mu_bc, xsq_dve=0,
                                tags=("s1", "s2")):
                """xt: accessor c -> [128, W] AP.  xsq on ACT Square (same
                table set as Ln/Exp) except the first `xsq_dve` tiles on
                DVE for balance."""
                s1 = ps_stat.tile([128, W], f32, tag=tags[0], name="s1")
                s2 = ps_stat.tile([128, W], f32, tag=tags[1], name="s2")
                for c in range(NCT):
                    xsq = wpool.tile([128, W], bf16, tag=f"xsq{c % 2}",
                                     name="xsq", bufs=1)
                    if c < xsq_dve:
                        nc.vector.tensor_tensor(xsq[:], xt(c), xt(c),
                                                alu.mult)
                    else:
                        nc.scalar.square(xsq[:], xt(c))
                    mmF(s1, ones_sb[:], xt(c), 0, W,
                        c == 0, c == NCT - 1)
                    mmF(s2, ones_sb[:], xsq, 0, W,
                        c == 0, c == NCT - 1)
                t_mu = spool.tile([128, W], f32, tag="t_mu", name="t_mu")
                t_m2 = spool.tile([128, W], f32, tag="t_m2", name="t_m2")
                t_v = spool.tile([128, W], f32, tag="t_mu", name="t_v")
                t_ln = spool.tile([128, W], f32, tag="t_m2", name="t_ln")
                nc.vector.tensor_scalar(t_mu[:], s1[:], 1.0 / C, None,
                                        alu.mult)
                nc.vector.tensor_tensor(t_m2[:], t_mu[:], t_mu[:], alu.mult)
                nc.vector.scalar_tensor_tensor(t_v[:], s2[:], 1.0 / C,
                                               t_m2[:], alu.mult,
                                               alu.subtract)
                nc.scalar.activation(t_ln[:], t_v[:], AF.Ln, bias=EPS)
                b1 = spool.tile([128, W], bf16, tag="b1", name="b1", bufs=2)
                nc.scalar.activation(b1[:], t_ln[:], AF.Exp, scale=-0.5)
                corr = spool.tile([2, W], bf16, tag="corr", name="corr",
                                  bufs=2)
                mu_bf = spool.tile([1, W], bf16, tag="mu_bf", name="mu_bf")
                nc.vector.tensor_scalar(mu_bf[:], s1[0:1, :], 1.0 / C, None,
                                        alu.mult)
                nc.vector.memset(corr[0:2, :], 1.0)
                nc.vector.tensor_tensor(corr[0:1, :], mu_bf[:], b1[0:1, :],
                                        alu.mult)
                mu_bc = None
                if want_mu_bc:
                    mu_bc = spool.tile([128, W], bf16, tag="mu_bc",
                                       name="mu_bc")
                    nc.vector.tensor_scalar(mu_bc[:], s1[:], 1.0 / C, None,
                                            alu.mult)
                return b1, corr, mu_bc

            # persistent small inputs (merged DMAs)
            WQKV = 2 * DP + DH
            wqkv_sb = cpool.tile([128, NCT * WQKV], bf16, tag="wqkv",
                                 name="wqkv")
            nc.sync.dma_start(wqkv_sb[:], wqkv)
            wq_sb = [wqkv_sb[:, c * WQKV:c * WQKV + DP] for c in range(NCT)]
            wk_sb = [wqkv_sb[:, c * WQKV + DP:c * WQKV + 2 * DP]
                     for c in range(NCT)]
            wv_sb = [wqkv_sb[:, c * WQKV + 2 * DP:(c + 1) * WQKV]
                     for c in range(NCT)]
            cq_sb = cpool.tile([2, DP], bf16, tag="cq", name="cq")
            ck_sb = cpool.tile([2, DP], bf16, tag="ck", name="ck")
            cv_sb = cpool.tile([2, DH], bf16, tag="cv", name="cv")
            nc.sync.dma_start(cq_sb[:], cq)
            nc.sync.dma_start(ck_sb[:], ck)
            nc.sync.dma_start(cv_sb[:], cv)
            ctx_sb = cpool.tile([128, NCT * CTX], bf16, tag="ctx", name="ctx")
            nc.sync.dma_start(ctx_sb[:], ctx_bf)
            ctxa = lambda c: ctx_sb[:, c * CTX:(c + 1) * CTX]
            cqc_sb = cpool.tile([2, C], bf16, tag="cqc", name="cqc")
            nc.sync.dma_start(cqc_sb[:], cqc)
            xo_sb = [rpool.tile([128, TG], f32, tag=f"xo{c}", name=f"xo{c}")
                     for c in range(NCT)]
            for c in range(NCT):
                nc.sync.dma_start(xo_sb[c][:], x_own[c * 128:(c + 1) * 128, :])

            x2 = [rpool.tile([128, TG], f32, tag=f"x2_{c}", name=f"x2_{c}")
                  for c in range(NCT)]
            x3 = [rpool.tile([128, TG], f32, tag=f"xo{c}", name=f"x3_{c}")
                  for c in range(NCT)]

            a2a_in = dpool.tile([H * DH, TG], bf16, name="a2a_in")
            a2a_out = dpool.tile([H * DH, TG], bf16, name="a2a_out")

            # ============== Phases A+B (pool scoped, freed after) ==========
            with tc.tile_pool(name="attn", bufs=1) as apool:
                cos_sb = [apool.tile([DP, MG], bf16, tag=f"cos{g}",
                                     name=f"cos{g}") for g in range(NMG)]
                sin_sb = [apool.tile([DP, MG], bf16, tag=f"sin{g}",
                                     name=f"sin{g}") for g in range(NMG)]
                # x is host-pre-tiled [128, NMG*NCT*MG]: one DMA per (b,g)
                xbf = [[apool.tile([128, NCT * MG], bf16, tag="xbf",
                                   name="xbf", bufs=2) for _ in range(NMG)]
                       for _ in range(B)]
                nc.sync.dma_start(xbf[0][0][:], x_bf[0][:, 0:NCT * MG])
                nc.sync.dma_start(ones_sb[:], ones_in)
                nc.sync.dma_start(tri_sb[:], tri)
                for g in range(NMG):
                    nc.sync.dma_start(cos_sb[g][:],
                                      cosT[:, g * MG:(g + 1) * MG])
                    nc.sync.dma_start(sin_sb[g][:],
                                      sinT[:, g * MG:(g + 1) * MG])
                nc.sync.dma_start(xbf[0][1][:],
                                  x_bf[0][:, NCT * MG:2 * NCT * MG])
                # x(1,0)/x(1,1) are issued inside the phase-A loop so their
                # buffer-reuse waits don't head-of-line-block the Sync queue

                q_sb = [[apool.tile([DP, MG], bf16, tag=f"q{bb}_{g}",
                                    name=f"q{bb}_{g}") for g in range(NMG)]
                        for bb in range(B)]
                k_sb = [[apool.tile([DP, MG], bf16, tag=f"k{bb}_{g}",
                                    name=f"k{bb}_{g}") for g in range(NMG)]
                        for bb in range(B)]
                v_sb = [[apool.tile([128, DH + 1], bf16, tag=f"v{bb}_{t}",
                                    name=f"v{bb}_{t}") for t in range(NKT)]
                        for bb in range(B)]
                o_sb = [[apool.tile([DH, MG], bf16, tag=f"o{bb}_{g}",
                                    name=f"o{bb}_{g}") for g in range(NMG)]
                        for bb in range(B)]

                # ---- Phase A: LN1 + QKV + RoPE ----
                with (
                    tc.tile_pool(name="ps_statA", bufs=1,
                                 space="PSUM") as ps_sA,
                    tc.tile_pool(name="ps_projA", bufs=2,
                                 space="PSUM") as ps_pA,
                ):
                    for bb in range(B):
                        for g in range(NMG):
                            if (bb, g) in ((0, 1), (1, 0)):
                                nb, ng = (1, 0) if g == 1 else (1, 1)
                                nc.sync.dma_start(
                                    xbf[nb][ng][:],
                                    x_bf[nb][:, ng * NCT * MG:
                                             (ng + 1) * NCT * MG])
                            xt = xbf[bb][g]
                            xa = lambda c: xt[:, c * MG:(c + 1) * MG]
                            b1, corr, _ = layernorm_stats(xa, ps_sA, MG,
                                                          False, xsq_dve=2)
                            z = [apool.tile([128, MG], bf16, tag=f"z{c}",
                                            name="z", bufs=2)
                                 for c in range(NCT)]
                            for c in range(NCT):
                                nc.vector.tensor_tensor(z[c][:], xa(c),
                                                        b1[:], alu.mult)
                            for wt, ct, dst in ((wq_sb, cq_sb, q_sb),
                                                (wk_sb, ck_sb, k_sb)):
                                ps = ps_pA.tile([DP, MG], f32, tag="proj",
                                                name="ps")
                                for c in range(NCT):
                                    mmF(ps, wt[c][:], z[c], 0, MG,
                                        c == 0, False)
                                mmF(ps, ct[:], corr, 0, MG, False, True)
                                raw = apool.tile([DP, MG], bf16, tag="qkraw",
                                                 name="raw", bufs=2)
                                nc.scalar.copy(raw[:], ps[:])
                                # RoPE: out[d] = raw[d]*cos + raw[d^64]*sin'
                                t1 = apool.tile([DP, MG], bf16, tag="rope1",
                                                name="t1", bufs=2)
                                t2 = apool.tile([DP, MG], bf16, tag="rope2",
                                                name="t2", bufs=2)
                                eng = nc.gpsimd if dst is q_sb else nc.vector
                                nc.vector.tensor_tensor(t1[:], raw[:],
                                                        cos_sb[g][:],
                                                        alu.mult)
                                eng.tensor_tensor(t2[0:64, :],
                                                  raw[64:128, :],
                                                  sin_sb[g][64:128, :],
                                                  alu.mult)
                                eng.tensor_tensor(t2[64:128, :],
                                                  raw[0:64, :],
                                                  sin_sb[g][0:64, :],
                                                  alu.mult)
                                nc.vector.tensor_tensor(dst[bb][g][:], t1[:],
                                                        t2[:], alu.add)
                            for tt in range(KPG):
                                kt = g * KPG + tt
                                ps = ps_pA.tile([128, DH], f32, tag="proj",
                                                name="ps")
                                for c in range(NCT):
                                    nc.tensor.matmul(
                                        ps[:],
                                        z[c][:, tt * 128:(tt + 1) * 128],
                                        wv_sb[c][:], start=(c == 0),
                                        stop=False)
                                nc.tensor.matmul(
                                    ps[:], corr[:, tt * 128:(tt + 1) * 128],
                                    cv_sb[:], start=False, stop=True)
                                nc.scalar.copy(v_sb[bb][kt][:, 0:DH],
                                               ps[:])
                                nc.vector.memset(v_sb[bb][kt][:, DH:DH + 1],
                                                 1.0)

                # ---- Phase B: causal attention ----
                with tc.tile_pool(name="ps_attn", bufs=1,
                                  space="PSUM") as ps_at:
                    with nc.allow_low_precision(
                            reason="softmax reciprocal bf16"):
                        for bb in range(B):
                            for qg in range(NMG):
                                o_ps = ps_at.tile([DH + 1, MG], f32, tag="av",
                                                  name="o_ps", bufs=1)
                                nkt = KPG * qg + KPG
                                for kt in range(nkt):
                                    p = kt - KPG * qg
                                    q0 = 0 if p < 0 else p * 128
                                    s_ps = ps_at.tile([128, MG], f32,
                                                      tag="scores",
                                                      name="s_ps", bufs=2)
                                    mmF(s_ps,
                                        k_sb[bb][kt // KPG][
                                            :, (kt % KPG) * 128:
                                            (kt % KPG + 1) * 128],
                                        q_sb[bb][qg], q0, MG, True, True)
                                    pt = apool.tile([128, MG], bf16,
                                                    tag="ptB", name="pt",
                                                    bufs=2)
                                    nc.scalar.activation(pt[:, q0:MG],
                                                         s_ps[:, q0:MG],
                                                         AF.Exp)
                                    if p >= 0:
                                        nc.vector.tensor_tensor(
                                            pt[:, q0:q0 + 128],
                                            pt[:, q0:q0 + 128],
                                            tri_sb[:], alu.mult)
                                    mmF(o_ps, v_sb[bb][kt][:], pt, q0, MG,
                                        kt == 0, kt == nkt - 1)
                                den = spool.tile([1, MG], f32, tag="den",
                                                 name="den", bufs=2)
                                nc.vector.tensor_copy(den[:],
                                                      o_ps[DH:DH + 1, :])
                                rcp = spool.tile([1, MG], f32, tag="rcp",
                                                 name="rcp", bufs=2)
                                nc.vector.reciprocal_approx_fast(rcp[:],
                                                                 den[:])
                                rcpb = spool.tile([1, MG], bf16, tag="rcpb",
                                                  name="rcpb", bufs=2)
                                nc.vector.tensor_copy(rcpb[:], rcp[:])
                                b_ps = ps_at.tile([128, MG], f32, tag="bcast",
                                                  name="b_ps", bufs=1)
                                mmF(b_ps, ones_sb[0:1, :], rcpb, 0, MG,
                                    True, True)
                                b_sb = apool.tile([128, MG], bf16, tag="bsb",
                                                  name="b_sb", bufs=2)
                                nc.vector.tensor_copy(b_sb[:], b_ps[:])
                                nc.vector.tensor_tensor(o_sb[bb][qg][:],
                                                        o_ps[0:DH, :],
                                                        b_sb[0:DH, :],
                                                        alu.mult)
                                for half in range(2):
                                    j = bb * 4 + qg * 2 + half
                                    nc.sync.dma_start(
                                        a2a_in[j * DH:(j + 1) * DH, :],
                                        o_sb[bb][qg][:, half * TG:
                                                     (half + 1) * TG])

            # fire the exchange; fill the wait with ctx-side cross-attn work
            if SKIP_COLLECTIVE:
                nc.sync.dma_start(a2a_out[:], a2a_in[:])
            else:
                nc.gpsimd.collective_compute(
                    "AllToAll", alu.bypass, replica_groups=RG8,
                    ins=[a2a_in.opt()], outs=[a2a_out.opt()])
            if debug:
                nc.sync.dma_start(dbg_ai, a2a_in[:])
                nc.sync.dma_start(dbg_ao, a2a_out[:])

            with (
                tc.tile_pool(name="cross", bufs=1) as xpool,
                tc.tile_pool(name="wstr", bufs=1) as wstr,
                tc.tile_pool(name="ps_projD", bufs=2, space="PSUM") as ps_pD,
                tc.tile_pool(name="ps_attn2", bufs=1, space="PSUM") as ps_at2,
            ):
                # ---- context-side projections (independent of the A2A) ----
                wkc_sb = xpool.tile([128, NCT * C], bf16, tag="wkc",
                                    name="wkc")
                wvc_sb = xpool.tile([128, NCT * C], bf16, tag="wvc",
                                    name="wvc")
                nc.sync.dma_start(wkc_sb[:], wkc)
                nc.sync.dma_start(wvc_sb[:], wvc)
                kc_sb = [xpool.tile([DH, CTX], bf16, tag=f"kc{h}",
                                    name=f"kc{h}") for h in range(H)]
                for h in range(H):
                    ps = ps_pD.tile([DH, CTX], f32, tag="proj", name="ps")
                    for c in range(NCT):
                        nc.tensor.matmul(
                            ps[:],
                            wkc_sb[:, c * C + h * DH:c * C + (h + 1) * DH],
                            ctxa(c), start=(c == 0), stop=(c == NCT - 1))
                    nc.vector.tensor_copy(kc_sb[h][:], ps[:])
                vc_sb = xpool.tile([128, H * (DH + 1)], bf16, tag="vc",
                                   name="vc")
                for half in range(2):
                    ps = ps_pD.tile([128, C // 2], f32, tag="proj", name="ps")
                    for c in range(NCT):
                        nc.tensor.matmul(
                            ps[:], ctxa(c),
                            wvc_sb[:, c * C + half * 384:
                                   c * C + (half + 1) * 384],
                            start=(c == 0), stop=(c == NCT - 1))
                    dv = vc_sb[:].rearrange("p (h d) -> p h d", h=H)[
                        :, half * 4:(half + 1) * 4, 0:DH]
                    sv = ps[:].rearrange("p (h d) -> p h d", h=4)
                    nc.vector.tensor_copy(dv, sv)
                nc.vector.memset(
                    vc_sb[:].rearrange("p (h d) -> p h d",
                                       h=H)[:, :, DH:DH + 1], 1.0)

                # ---- after the A2A: gather heads, out-project, residual ----
                oa = [xpool.tile([128, TG], bf16, tag=f"oa{k}",
                                 name=f"oa{k}") for k in range(NCT)]
                for k in range(NCT):
                    nc.sync.dma_start(oa[k][:],
                                      a2a_out[k * 128:(k + 1) * 128, :])
                x2bf = [xpool.tile([128, TG], bf16, tag=f"x2bf{c}",
                                   name=f"x2bf{c}") for c in range(NCT)]
                for ot in range(NCT):
                    wot = wstr.tile([128, NCT * 128], bf16, tag="wot",
                                    name="wot", bufs=3)
                    nc.sync.dma_start(wot[:],
                                      wo_t[:, ot * C:(ot + 1) * C])
                    ps = ps_pD.tile([128, TG], f32, tag="proj", name="ps")
                    for k in range(NCT):
                        nc.tensor.matmul(ps[:],
                                         wot[:, k * 128:(k + 1) * 128],
                                         oa[k][:], start=(k == 0),
                                         stop=(k == NCT - 1))
                    nc.vector.tensor_tensor(x2[ot][:], ps[:], xo_sb[ot][:],
                                            alu.add)
                    nc.scalar.copy(x2bf[ot][:], x2[ot][:])
                    if debug:
                        nc.sync.dma_start(dbg_x2[ot * 128:(ot + 1) * 128, :],
                                          x2[ot][:])

                # ---- LN2 + cross-attention ----
                b1, corr, _ = layernorm_stats(lambda c: x2bf[c][:], ps_pD,
                                              TG, False, tags=("proj", "proj"))
                z2 = [xpool.tile([128, TG], bf16, tag=f"z2_{c}",
                                 name=f"z2_{c}") for c in range(NCT)]
                for c in range(NCT):
                    nc.vector.tensor_tensor(z2[c][:], x2bf[c][:], b1[:],
                                            alu.mult)
                ocfm = [xpool.tile([128, TG], bf16, tag=f"oa{c}",
                                   name=f"ocfm{c}") for c in range(NCT)]
                wqct = [wstr.tile([128, NCT * DH], bf16, tag="wqct",
                                  name="wqct", bufs=4) for h in range(H)]
                for h in range(H):
                    nc.sync.dma_start(
                        wqct[h][:],
                        wqc_t[:, h * NCT * DH:(h + 1) * NCT * DH])
                with nc.allow_low_precision(reason="softmax reciprocal bf16"):
                    for h in range(H):
                        qc_ps = ps_pD.tile([DH, TG], f32, tag="proj",
                                           name="ps")
                        for c in range(NCT):
                            nc.tensor.matmul(
                                qc_ps[:], wqct[h][:, c * DH:(c + 1) * DH],
                                z2[c][:], start=(c == 0), stop=False)
                        nc.tensor.matmul(qc_ps[:],
                                         cqc_sb[:, h * DH:(h + 1) * DH],
                                         corr[:], start=False, stop=True)
                        qc = wpool.tile([DH, TG], bf16, tag="qc", name="qc")
                        nc.vector.tensor_copy(qc[:], qc_ps[:])
                        s_ps = ps_at2.tile([CTX, TG], f32, tag="scores",
                                           name="s_ps", bufs=2)
                        nc.tensor.matmul(s_ps[:], kc_sb[h][:], qc[:],
                                         start=True, stop=True)
                        pt = wpool.tile([CTX, TG], bf16, tag="ptD",
                                        name="pt")
                        nc.scalar.activation(pt[:], s_ps[:], AF.Exp)
                        o_ps = ps_at2.tile([DH + 1, TG], f32, tag="av",
                                           name="o_ps", bufs=2)
                        nc.tensor.matmul(
                            o_ps[:],
                            vc_sb[:, h * (DH + 1):(h + 1) * (DH + 1)],
                            pt[:], start=True, stop=True)
                        den = spool.tile([1, TG], f32, tag="den", name="den",
                                         bufs=2)
                        nc.scalar.copy(den[:], o_ps[DH:DH + 1, :])
                        rcp = spool.tile([1, TG], f32, tag="rcp", name="rcp",
                                         bufs=2)
                        nc.vector.reciprocal_approx_fast(rcp[:], den[:])
                        rcpb = spool.tile([1, TG], bf16, tag="rcpb",
                                          name="rcpb", bufs=2)
                        nc.vector.tensor_copy(rcpb[:], rcp[:])
                        b_ps = ps_at2.tile([128, TG], f32, tag="bcast",
                                           name="b_ps", bufs=1)
                        nc.tensor.matmul(b_ps[:], ones_sb[0:1, :], rcpb[:],
                                         start=True, stop=True)
                        b_sb = wpool.tile([128, TG], bf16, tag="bsbD",
                                          name="b_sb")
                        nc.scalar.copy(b_sb[:], b_ps[:])

                        def _maxn(v):
                            if v % 128 == 0:
                                return 128
                            if v % 64 == 0:
                                return 64
                            return 32
                        pos = 0
                        while pos < DH:
                            r = h * DH + pos
                            c0, off = r // 128, r % 128
                            n = min(_maxn(off), _maxn(pos), DH - pos,
                                    128 - off)
                            nc.vector.tensor_tensor(
                                ocfm[c0][off:off + n, :],
                                o_ps[pos:pos + n, :],
                                b_sb[pos:pos + n, :], alu.mult)
                            pos += n

                x3bf = [xpool.tile([128, TG], bf16, tag=f"z2_{c}",
                                   name=f"x3bf{c}") for c in range(NCT)]
                for ot in range(NCT):
                    wcot = wstr.tile([128, NCT * 128], bf16, tag="wcot",
                                     name="wcot", bufs=3)
                    nc.sync.dma_start(wcot[:],
                                      wco_t[:, ot * C:(ot + 1) * C])
                    ps = ps_pD.tile([128, TG], f32, tag="proj", name="ps")
                    for c in range(NCT):
                        nc.tensor.matmul(ps[:],
                                         wcot[:, c * 128:(c + 1) * 128],
                                         ocfm[c][:], start=(c == 0),
                                         stop=(c == NCT - 1))
                    nc.vector.tensor_tensor(x3[ot][:], ps[:], x2[ot][:],
                                            alu.add)
                    nc.scalar.copy(x3bf[ot][:], x3[ot][:])
                    if debug:
                        nc.sync.dma_start(dbg_x3[ot * 128:(ot + 1) * 128, :],
                                          x3[ot][:])

                # ---- LN3 (z3 mean-subtracted on DVE, fp8 DoubleRow pack) --
                b1, corr, mu_bc = layernorm_stats(lambda c: x3bf[c][:],
                                                  ps_pD, TG, True,
                                                  tags=("proj", "proj"))
                z3p = [xpool.tile([128, 2 * TG], fp8, tag=f"z3p{cp}",
                                  name=f"z3p{cp}") for cp in range(3)]
                for c in range(NCT):
                    zt = wpool.tile([128, TG], bf16, tag="zt", name="zt")
                    nc.vector.tensor_tensor(zt[:], x3bf[c][:], mu_bc[:],
                                            alu.subtract)
                    dst = z3p[c // 2][:, (c % 2) * TG:(c % 2 + 1) * TG]
                    nc.vector.tensor_tensor(dst, zt[:], b1[:], alu.mult)
                if not bias_zero:
                    onerow = xpool.tile([1, TG], bf16, tag="onerow",
                                        name="onerow")
                    nc.vector.memset(onerow[:], 1.0)
                    cg_sb = xpool.tile([1, II], bf16, tag="cg", name="cg")
                    cu_sb = xpool.tile([1, II], bf16, tag="cu", name="cu")
                    nc.sync.dma_start(cg_sb[:], cg)
                    nc.sync.dma_start(cu_sb[:], cu)

                # ---- Phase E: SwiGLU FFN, fp8 DoubleRow (weights x64) ----
                hh = [xpool.tile([128, 2 * TG], fp8, tag=f"hh{ip}",
                                 name=f"hh{ip}") for ip in range(12)]
                DR = mybir.MatmulPerfMode.DoubleRow
                if True:
                    for it in range(NIT):
                        wgt = wstr.tile([128, 3 * 256], fp8, tag="wgt",
                                        name="wgt", bufs=3)
                        wut = wstr.tile([128, 3 * 256], fp8, tag="wut",
                                        name="wut", bufs=3)
                        nc.sync.dma_start(
                            wgt[:], wg_t[:, it * 768:(it + 1) * 768])
                        nc.sync.dma_start(
                            wut[:], wu_t[:, it * 768:(it + 1) * 768])
                        g_ps = ps_at2.tile([128, TG], f32, tag="scores",
                                           name="g_ps", bufs=2)
                        u_ps = ps_pD.tile([128, TG], f32, tag="proj",
                                          name="u_ps")
                        for cp in range(3):
                            last = (cp == 2) and bias_zero
                            zr = z3p[cp][:].rearrange("p (r t) -> p r t",
                                                      r=2)
                            for w_, ps_ in ((wgt, g_ps), (wut, u_ps)):
                                wr = w_[:, cp * 256:(cp + 1) * 256].rearrange(
                                    "p (r m) -> p r m", r=2)
                                nc.tensor.matmul(ps_[:], wr, zr,
                                                 start=(cp == 0), stop=last,
                                                 perf_mode=DR)
                        if not bias_zero:
                            nc.tensor.matmul(
                                g_ps[:], cg_sb[:, it * 128:(it + 1) * 128],
                                onerow[:], start=False, stop=True)
                            nc.tensor.matmul(
                                u_ps[:], cu_sb[:, it * 128:(it + 1) * 128],
                                onerow[:], start=False, stop=True)
                        # silu(g_true)*u_true: ACT Silu descales g (x1/64),
                        # the u descale (1/64) folds into the hh write
                        sg = wpool.tile([128, TG], bf16, tag="sg", name="sg")
                        nc.scalar.activation(sg[:], g_ps[:], AF.Silu,
                                             scale=1.0 / 64)
                        hdst = hh[it // 2][:, (it % 2) * TG:
                                           (it % 2 + 1) * TG]
                        nc.vector.scalar_tensor_tensor(hdst, sg[:],
                                                       1.0 / 64, u_ps[:],
                                                       alu.mult, alu.mult)
                if True:
                    for ot in range(NCT):
                        wdt = wstr.tile([128, 12 * 256], fp8, tag="wdt",
                                        name="wdt", bufs=2)
                        nc.sync.dma_start(
                            wdt[:], wd_t[:, ot * 12 * 256:
                                         (ot + 1) * 12 * 256])
                        d_ps = ps_at2.tile([128, TG], f32, tag="scores",
                                           name="d_ps", bufs=2)
                        for ip in range(12):
                            wr = wdt[:, ip * 256:(ip + 1) * 256].rearrange(
                                "p (r m) -> p r m", r=2)
                            hr = hh[ip][:].rearrange("p (r t) -> p r t", r=2)
                            nc.tensor.matmul(d_ps[:], wr, hr,
                                             start=(ip == 0), stop=(ip == 11),
                                             perf_mode=DR)
                        xf = wpool.tile([128, TG], f32, tag="xf", name="xf")
                        nc.vector.scalar_tensor_tensor(xf[:], d_ps[:],
                                                       1.0 / 64, x3[ot][:],
                                                       alu.mult, alu.add)
                        nc.sync.dma_start(out_x[ot * 128:(ot + 1) * 128, :],
                                          xf[:])

    nc.compile()
    return nc


def _rope_tables(head_dim, height, width, frames, base=10000.0):
    d = head_dim // 3
    dx, dy, dt_ = d, d, head_dim - 2 * d

    def freqs(n, dd):
        inv = 1.0 / base ** (np.arange(0, dd, 2, dtype=np.float32) / dd)
        f = np.outer(np.arange(n, dtype=np.float32), inv)
        return np.concatenate([f, f], axis=-1)

    fx, fy, ft = freqs(width, dx), freqs(height, dy), freqs(frames, dt_)
    shp = (frames, height, width)
    cx = np.broadcast_to(np.cos(fx)[None, None, :, :], shp + (dx,))
    sx = np.broadcast_to(np.sin(fx)[None, None, :, :], shp + (dx,))
    cy = np.broadcast_to(np.cos(fy)[None, :, None, :], shp + (dy,))
    sy = np.broadcast_to(np.sin(fy)[None, :, None, :], shp + (dy,))
    ct = np.broadcast_to(np.cos(ft)[:, None, None, :], shp + (dt_,))
    st = np.broadcast_to(np.sin(ft)[:, None, None, :], shp + (dt_,))
    cos = np.concatenate([cx, cy, ct], axis=-1).reshape(-1, head_dim)
    sin = np.concatenate([sx, sy, st], axis=-1).reshape(-1, head_dim)
    return cos.astype(np.float32), sin.astype(np.float32)


def _qk_perm():
    """Stored-index -> original head-dim map (-1 = zero pad), length 128.
    Layout [x1(48) pad16 | x2(48) pad16] puts every rotate-half partner at
    stored index s^64."""
    P = np.full(DP, -1, np.int64)
    P[0:48] = np.arange(0, 48)
    P[64:112] = np.arange(48, 96)
    return P


def _tile6(W, nb):
    """[C, nb*128] -> [128, nb*NCT*128] with block (b, c) at cols
    (b*NCT+c)*128."""
    return np.ascontiguousarray(
        W.reshape(NCT, 128, nb, 128).transpose(1, 2, 0, 3).reshape(
            128, nb * NCT * 128))


def _prep_inputs(inputs):
    """Host-side prep.  Returns (bias_zero, in_maps)."""
    f = lambda k: np.asarray(inputs[k], np.float32)
    x, context = f("x"), f("context")
    wqkv, w_attn_out = f("wqkv"), f("w_attn_out")
    ln1_g, ln1_b = f("ln1_g"), f("ln1_b")
    wq_c, wk_c, wv_c, w_cross_out = (f("wq_c"), f("wk_c"), f("wv_c"),
                                     f("w_cross_out"))
    ln2_g, ln2_b = f("ln2_g"), f("ln2_b")
    w_gate, w_up, w_down = f("w_gate"), f("w_up"), f("w_down")
    ln3_g, ln3_b = f("ln3_g"), f("ln3_b")
    height, width, frames = (int(inputs["height"]), int(inputs["width"]),
                             int(inputs["frames"]))

    bias_zero = bool((ln3_b == 0).all())
    sc = DH ** -0.25

    def fold(W, g, b, scale=1.0):
        Wg = g[:, None] * W * scale
        c0 = -Wg.sum(axis=0)
        c1 = b @ W * scale
        return Wg, np.stack([c0, c1]).astype(BF16)

    wqkv_g, cqkv = fold(wqkv, ln1_g, ln1_b)
    wqkv_g[:, :C] *= sc
    wqkv_g[:, C:2 * C] *= sc
    cqkv[:, :2 * C] *= BF16(sc)
    wqc_g, cqc = fold(wq_c, ln2_g, ln2_b, sc)
    wkc_s = (wk_c * sc).astype(BF16)
    # LN3: mean handled on-chip; fold only the gain.
    wg_g = (ln3_g[:, None] * w_gate).astype(BF16)
    wu_g = (ln3_g[:, None] * w_up).astype(BF16)

    cos, sin = _rope_tables(DH, height, width, frames)
    sinp = sin.copy()
    sinp[:, :DH // 2] *= -1.0
    P = _qk_perm()
    valid = P >= 0
    Pc = np.where(valid, P, 0)
    cosP = np.where(valid[None, :], cos[:, Pc], 0.0)
    sinP = np.where(valid[None, :], sinp[:, Pc], 0.0)
    cosT = np.ascontiguousarray(cosP.T).astype(BF16)
    # sin is read at raw's partition base (SB inputs must share it), so
    # pre-swap columns: sin_sb[d] = sinP[d^64], giving
    # out[d] = raw[d]*cosP[d] + raw[d^64]*sin_sb[d^64] = ... + raw[d^64]*sinP[d]
    sinT = np.ascontiguousarray(sinP[:, np.arange(DP) ^ 64].T).astype(BF16)

    def permute_qk(Wh):  # [rows, DH] -> [rows, DP] permuted+padded
        out = np.zeros((Wh.shape[0], DP), Wh.dtype)
        out[:, valid] = Wh[:, Pc[valid]]
        return out

    tri = np.triu(np.ones((128, 128), np.float32)).astype(BF16)
    ones128 = np.ones((128, 128), np.float32).astype(BF16)

    xT = np.ascontiguousarray(x.transpose(0, 2, 1))          # [B, C, S]
    ctxT = np.ascontiguousarray(context.transpose(0, 2, 1))  # [B, C, CTX]

    # pre-tiled streamed weights (shared across cores)
    wqc_tl = np.ascontiguousarray(
        wqc_g.astype(BF16).reshape(NCT, 128, H, DH).transpose(
            1, 2, 0, 3).reshape(128, H * NCT * DH))
    wo_tl = _tile6(w_attn_out.astype(BF16), NCT)
    wco_tl = _tile6(w_cross_out.astype(BF16), NCT)
    FP8 = ml_dtypes.float8_e4m3fn

    def pack_dr(W, nb):
        # [K, nb*128] -> [128, nb*(K/256)*256] fp8 DoubleRow blocks.
        # Slot (p, parity r) holds contraction row kp*256 + r*128 + p,
        # matching how the kernel packs z3/hh pairs on-chip:
        # lhsT[p, ((b*KP + kp)*128 + m)*2 + r] = W[kp*256 + r*128 + p, b*128+m]
        K = W.shape[0]
        KP = K // 256
        t = W.reshape(KP, 2, 128, nb, 128)          # [kp, r, p, b, m]
        t = t.transpose(2, 3, 0, 1, 4)              # [p, b, kp, r, m]
        return np.ascontiguousarray(t.reshape(128, nb * KP * 256)).astype(FP8)

    wg_tl = pack_dr(np.float32(64.0) * wg_g.astype(np.float32), NIT)
    wu_tl = pack_dr(np.float32(64.0) * wu_g.astype(np.float32), NIT)
    wd_tl = pack_dr(np.float32(64.0) * w_down, NCT)

    def xtile(xb):  # [C, S] -> [128, NMG*NCT*MG], block (g, c)
        return np.ascontiguousarray(
            xb.reshape(NCT, 128, NMG, MG).transpose(1, 2, 0, 3).reshape(
                128, NMG * NCT * MG))

    def rowtile(W, w):  # [C, w] -> [128, NCT*w], block c at cols c*w
        return np.ascontiguousarray(
            W.reshape(NCT, 128, w).transpose(1, 0, 2).reshape(128, NCT * w))

    shared = dict(
        cosT=cosT, sinT=sinT, tri=tri, ones_in=ones128,
        x_bf0=xtile(xT[0].astype(BF16)), x_bf1=xtile(xT[1].astype(BF16)),
        wo_t=wo_tl, wqc_t=wqc_tl, wco_t=wco_tl,
        wkc=rowtile(wkc_s, C), wvc=rowtile(wv_c.astype(BF16), C),
        wg_t=wg_tl, wu_t=wu_tl, wd_t=wd_tl,
        cqc=cqc,
    )
    if not bias_zero:
        shared["cg"] = (ln3_b @ w_gate).astype(BF16)[None, :]
        shared["cu"] = (ln3_b @ w_up).astype(BF16)[None, :]
    in_maps = []
    for core in range(NCORES):
        h = core                      # head owned in phases A/B
        b, gq = core // 4, core % 4   # batch/token-group in phases D/E
        m = dict(shared)
        m["x_own"] = np.ascontiguousarray(xT[b][:, gq * TG:(gq + 1) * TG])
        m["ctx_bf"] = rowtile(ctxT[b].astype(BF16), CTX)
        qs = slice(DH * h, DH * (h + 1))
        wq_h = permute_qk(wqkv_g[:, :C][:, qs]).astype(BF16)
        wk_h = permute_qk(wqkv_g[:, C:2 * C][:, qs]).astype(BF16)
        wv_h = wqkv_g[:, 2 * C:][:, qs].astype(BF16)
        # merged [wq|wk|wv] per 128-row tile: [128, NCT*(2*DP+DH)]
        wqkv_h = np.concatenate([wq_h, wk_h, wv_h], axis=1)  # [C, 352]
        m["wqkv_t"] = np.ascontiguousarray(
            wqkv_h.reshape(NCT, 128, 2 * DP + DH).transpose(1, 0, 2).reshape(
                128, NCT * (2 * DP + DH)))
        m["cq"] = permute_qk(cqkv[:, :C][:, qs].astype(np.float32)
                             ).astype(BF16)
        m["ck"] = permute_qk(cqkv[:, C:2 * C][:, qs].astype(np.float32)
                             ).astype(BF16)
        m["cv"] = np.ascontiguousarray(cqkv[:, 2 * C:][:, qs])
        in_maps.append(m)
    return bias_zero, in_maps


def _get_nc(inputs):
    bias_zero, in_maps = _prep_inputs(inputs)
    key = ("nc", bias_zero)
    if key not in _CACHE:
        _CACHE[key] = _build_program(bias_zero)
    return _CACHE[key], in_maps


def kernel(**inputs):
    from concourse import bass_utils
    nc, in_maps = _get_nc(inputs)
    res = bass_utils.run_bass_kernel_spmd(nc, in_maps,
                                          core_ids=list(range(NCORES)))
    out = np.empty((B, C, S), np.float32)
    for core in range(NCORES):
        b, g = core // 4, core % 4
        out[b][:, g * TG:(g + 1) * TG] = res.results[core]["out_x"]
    return np.ascontiguousarray(out.transpose(0, 2, 1))
